# revision 7
# baseline (speedup 1.0000x reference)
"""DynamicSegmentationHead Trainium2 kernel, restructured for overlap.

Data-parallel over 16 clip-frames: each core handles 2 frames (100 queries).
Query order is host-permuted to group-major n' = [f0 q0-44, f1 q0-44,
f0 q45-49, f1 q45-49] so every weight-scatter is one uniform strided DMA
and the L3 psum block maps to output rows with 3 clean DMAs per flush.

Controller runs in fp16 over an extended K of 771: host supplies
[hs; hs*refx; hs*refy; ones; refx; refy] so the per-query constant row
c = w8*refx + w9*refy + b0 falls out of the matmul directly (no separate
c-row pipeline). pw23 -> pw1 -> pw0 ordering releases the T3/T2/T1 build
paths in deadline order.

Per core, per 512-pixel chunk (17 chunks):
  L1: y1 = relu(T1 @ Ft)    7 matmuls K=11/22, M=120/80, psum rot x3
  L2: y2 = relu(T2 @ y1)    7 matmuls K=121 blockdiag, psum rot x3
  L3: one [100,512] psum, 7 accumulating matmuls, emitted with a
      progressive 2-chunk skew (first chains deferred to iteration 4 so
      they never wait on the T3 scatter DMAs), Y2 5-deep
  ACT stage copy + output flush DMAs per span, relus split ACT/DVE.

Steady state is ACT/DVE-bound: 15 psum-evacuation ops per chunk across
2 engines is the structural floor (PSUM has 8 banks; DMA cannot read
PSUM; 7 query groups is minimal since 800 M-rows / 128 partitions).
"""

import numpy as np

import concourse.bass as bass
import concourse.bacc as bacc
import concourse.tile as tile
from concourse import mybir
from concourse import bass_utils

F32 = mybir.dt.float32
F32R = mybir.dt.float32r
F16 = mybir.dt.float16

HID = 256
Q = 50
H, W = 72, 120
P = H * W            # 8640
NQ = 100             # queries per core (2 frames)
NCORES = 8
STRIDE = 4

CHUNK = 512
NCH = 17
CHUNKS = [(i * CHUNK, CHUNK) for i in range(16)] + [(16 * CHUNK, P - 16 * CHUNK)]
QUARTERS = [(0, 4), (4, 8), (8, 12), (12, 16), (16, 17)]   # flush spans

# groups in n' (group-major) order: 3 f0, 3 f1, 1 mixed
# (band, k1, nq); n'-base of group g is 15*g
GROUPS = [(0, 11, 15)] * 3 + [(32, 11, 15)] * 3 + [(64, 22, 10)]

Relu = mybir.ActivationFunctionType.Relu
Copy = mybir.ActivationFunctionType.Copy


def _build_program():
    nc = bacc.Bacc("TRN2", target_bir_lowering=False, debug=False)
    R = lambda ap: ap.bitcast(F32R)
    FV = lambda ap: ap.bitcast(F32)

    mf = nc.dram_tensor("mf", [2, 8, P], F32, kind="ExternalInput").ap()
    # hsall: 7 k-chunks of [hs(256); hs*rx(256); hs*ry(256); 1; rx; ry],
    # packed column-major in chunk order [0, 1, 6, 2, 3, 4, 5] so the
    # L2/L3-critical chunks (0, 1, 6) transfer first.
    hsall = nc.dram_tensor("hsall", [128, 700], F16, kind="ExternalInput").ap()
    wall = nc.dram_tensor("wall", [128, 1183], F16, kind="ExternalInput").ap()
    cst = nc.dram_tensor("cst", [3, P], F32, kind="ExternalInput").ap()
    outp = nc.dram_tensor("outp", [NQ, P], F32, kind="ExternalOutput").ap()

    def relu_act(out_ap, in_ap):
        nc.scalar.activation(out_ap, in_ap, Relu)

    def relu_dve(out_ap, in_ap):
        nc.vector.tensor_scalar(out=out_ap, in0=in_ap, scalar1=0.0,
                                scalar2=None, op0=mybir.AluOpType.max)

    with tile.TileContext(nc) as tc:
        with tc.tile_pool(name="persist", bufs=1) as pers:
            # ---------------- persistent SBUF ----------------
            Ft = pers.tile([128, P], F32R, tag="F")
            HSB = pers.tile([128, 700], F16, tag="HSB")
            WSB = pers.tile([128, 1183], F16, tag="WSB")
            PW0S = pers.tile([11, 800], F32, tag="PW0S")
            PW1S = pers.tile([9, 840], F32, tag="PW1S")
            PW23S = pers.tile([9, 105], F32, tag="PW23S")

            T1 = pers.tile([86, 840], F32R, tag="T1")
            T2 = pers.tile([121, 840], F32R, tag="T2")
            T3 = pers.tile([121, 705], F32R, tag="T3")
            Y1 = pers.tile([121, 7168], F32R, tag="Y1")   # (g,buf) col blocks
            Y2 = pers.tile([121, 17920], F32R, tag="Y2")
            STG = [pers.tile([NQ, 2048], F32, name=f"STG{i}", tag=f"STG{i}")
                   for i in range(2)]

            pstr = lambda t: t.ap[0][0]   # partition stride (elements)

            def y1slice(g, buf, cl, rows=121):
                base = (2 * g + buf) * CHUNK
                return Y1[0:rows, base:base + cl]

            def y2slice(g, buf, cl, rows=121):
                base = (5 * g + buf) * CHUNK
                return Y2[0:rows, base:base + cl]

            # ---------------- input DMAs ----------------
            nc.sync.dma_start(out=HSB[:, 0:300], in_=hsall[:, 0:300])
            nc.scalar.dma_start(out=WSB[:, 0:507], in_=wall[:, 0:507])
            nc.sync.dma_start(out=HSB[:, 300:700], in_=hsall[:, 300:700])
            nc.scalar.dma_start(out=WSB[:, 507:1183], in_=wall[:, 507:1183])

            # F layout: [0:8]=feat0, [8:10]=-gx,-gy, [10]=ones,
            #           [32:40]=feat1, [40:42]=-gx,-gy, [42]=ones,
            #           [64:72]=feat0, [72:74]=-gx,-gy, [74]=ones,
            #           [75:83]=feat1, [83:85]=-gx,-gy, [85]=ones
            nc.sync.dma_start(out=R(Ft[0:8, :]), in_=R(mf[0]))
            nc.sync.dma_start(out=R(Ft[8:11, :]), in_=R(cst[0:3, :]))
            nc.scalar.dma_start(out=R(Ft[32:40, :]), in_=R(mf[1]))
            nc.scalar.dma_start(out=R(Ft[40:43, :]), in_=R(cst[0:3, :]))
            # mixed band: SBUF->SBUF copies of the two main bands
            with tc.high_priority():
                nc.gpsimd.dma_start(out=R(Ft[64:75, :]), in_=R(Ft[0:11, :]))
                nc.gpsimd.dma_start(out=R(Ft[75:86, :]), in_=R(Ft[32:43, :]))
                # ones rows of Y1/Y2 (row 120) from the Ft ones row
                nc.gpsimd.dma_start(out=FV(Y1[120:121, :]),
                                    in_=FV(Ft[10:11, 0:7168]))
                nc.gpsimd.dma_start(out=FV(Y2[120:121, 0:8640]),
                                    in_=FV(Ft[10:11, 0:8640]))
                nc.gpsimd.dma_start(out=FV(Y2[120:121, 8640:17280]),
                                    in_=FV(Ft[10:11, 0:8640]))
                nc.gpsimd.dma_start(out=FV(Y2[120:121, 17280:17920]),
                                    in_=FV(Ft[10:11, 0:640]))

            # ---------------- zero fills ----------------
            nc.gpsimd.memset(FV(T2[:, :]), 0.0)
            nc.gpsimd.memset(FV(T3[:, :]), 0.0)
            nc.gpsimd.memset(FV(T1[64:86, 720:840]), 0.0)
            nc.gpsimd.memset(PW1S[0:9, 800:840], 0.0)
            nc.gpsimd.memset(PW23S[0:9, 100:105], 0.0)
            # mixed-group stale rows (80-119) of Y1/Y2 col blocks
            nc.gpsimd.memset(FV(Y1[64:120, 6144:7168]), 0.0)
            nc.gpsimd.memset(FV(Y2[64:120, 15360:17920]), 0.0)

            # ---------------- controller matmuls (fp16) ----------------
            with tc.tile_pool(name="psctrl", bufs=1, space="PSUM") as psc:
                pw1p = psc.tile([9, 1024], F32, tag="pw1p")
                pw23p = psc.tile([9, NQ], F32, tag="pw23p")
                pw0p = psc.tile([11, 1024], F32, tag="pw0p")
                # packed slot s holds logical chunk [0,1,6,2,3,4,5][s]
                # L2/L3 params contract over hs + tail chunks (slots 0,1,2)
                k3 = [(0, 128), (1, 128), (2, 3)]
                # L1 params (incl c row) contract over all 7 slots
                k7 = [(0, 128), (1, 128), (3, 128), (4, 128), (5, 128),
                      (6, 128), (2, 3)]
                # w1/b1 params first: T2 build is the long pole
                for i, (kc, kn) in enumerate(k3):
                    c0 = kc * 169 + 160
                    nc.tensor.matmul(
                        pw23p[0:9, 0:NQ],
                        WSB[0:kn, c0:c0 + 9],
                        HSB[0:kn, kc * 100:kc * 100 + NQ],
                        start=(i == 0), stop=(i == 2))
                for o2 in range(8):
                    for i, (kc, kn) in enumerate(k3):
                        c0 = kc * 169 + 88 + o2 * 9
                        nc.tensor.matmul(
                            pw1p[0:9, o2 * 128:o2 * 128 + NQ],
                            WSB[0:kn, c0:c0 + 9],
                            HSB[0:kn, kc * 100:kc * 100 + NQ],
                            start=(i == 0), stop=(i == 2))
                for o in range(8):
                    for i, (kc, kn) in enumerate(k7):
                        c0 = kc * 169 + o * 11
                        nc.tensor.matmul(
                            pw0p[0:11, o * 128:o * 128 + NQ],
                            WSB[0:kn, c0:c0 + 11],
                            HSB[0:kn, kc * 100:kc * 100 + NQ],
                            start=(i == 0), stop=(i == 6))

                # evacuate psum -> SBUF (the only way out of PSUM)
                nc.vector.tensor_copy(
                    PW1S[:, 0:800].rearrange("p (n o) -> p o n", o=8),
                    pw1p.rearrange("p (o n) -> p o n", o=8)[:, :, 0:NQ])
                nc.scalar.activation(PW23S[:, 0:NQ], pw23p[:, :], Copy)
                nc.scalar.activation(
                    PW0S.rearrange("p (o n) -> p o n", o=8),
                    pw0p.rearrange("p (o n) -> p o n", o=8)[:, :, 0:NQ], Copy)

            # ---------------- weight scatters ----------------
            # T2 per-j: dst [j*8:(j+1)*8, g*120+j*8+o2], src PW1S[o, (15g+j)*8+o2]
            dma_rot = [nc.sync, nc.sync, nc.sync, nc.gpsimd]
            for j in range(15):
                gcnt = 7 if j < 10 else 6
                if j % 4 == 0:
                    # 32-aligned partition base: engine copy
                    dst = T2[j * 8:j * 8 + 8, :].rearrange(
                        "p (g x) -> p g x", g=7)[:, 0:gcnt, j * 8:j * 8 + 8]
                    sv = PW1S[0:8, 0:840].rearrange(
                        "p (g x) -> p g x", g=7)[:, 0:gcnt, j * 8:j * 8 + 8]
                    if j % 8 == 0:
                        nc.vector.tensor_copy(dst, sv)
                    else:
                        nc.scalar.activation(dst, sv, Copy)
                else:
                    s = bass.AP(tensor=PW1S.tensor, offset=PW1S.offset + j * 8,
                                ap=[[pstr(PW1S), 8], [120, gcnt], [1, 8]])
                    dst = bass.AP(tensor=T2.tensor,
                                  offset=T2.offset + j * 8 * pstr(T2) + j * 8,
                                  ap=[[pstr(T2), 8], [120, gcnt], [1, 8]])
                    dma_rot[j % 4].dma_start(out=dst.bitcast(F32R),
                                             in_=s.bitcast(F32R))
            # T2 b1 row: dst [120, g*120+j*8+o2] <- PW1S[8, (15g+j)*8+o2]
            srcb1 = bass.AP(tensor=PW1S.tensor, offset=PW1S.offset + 8 * pstr(PW1S),
                            ap=[[pstr(PW1S), 1], [120, 7], [1, 120]])
            dstb1 = bass.AP(tensor=T2.tensor, offset=T2.offset + 120 * pstr(T2),
                            ap=[[pstr(T2), 1], [120, 7], [1, 120]])
            nc.sync.dma_start(out=dstb1.bitcast(F32R), in_=srcb1.bitcast(F32R))

            # T1 scatters (engine copies where partition base allows)
            # f0 block: rows 0-9 <- PW0S rows 0-9, cols (g,j,o) g-major
            src_f0 = bass.AP(tensor=PW0S.tensor, offset=PW0S.offset,
                             ap=[[pstr(PW0S), 11], [1, 45], [100, 8]])
            nc.vector.tensor_copy(T1[0:11, 0:360].rearrange(
                "p (n o) -> p n o", o=8), src_f0)
            src_f1 = bass.AP(tensor=PW0S.tensor, offset=PW0S.offset + 45,
                             ap=[[pstr(PW0S), 11], [1, 45], [100, 8]])
            nc.scalar.activation(T1[32:43, 360:720].rearrange(
                "p (n o) -> p n o", o=8), src_f1, Copy)
            # mixed f0-half rows 64-74 incl c (n' 90-94), base 64 ok
            src_m0 = bass.AP(tensor=PW0S.tensor, offset=PW0S.offset + 90,
                             ap=[[pstr(PW0S), 11], [1, 5], [100, 8]])
            nc.vector.tensor_copy(T1[64:75, 720:760].rearrange(
                "p (n o) -> p n o", o=8), src_m0)
            # mixed f1-half rows 75-85 incl c (n' 95-99): base 75 -> DMA/query
            for n in range(5):
                src_m1 = bass.AP(tensor=PW0S.tensor, offset=PW0S.offset + 95 + n,
                                 ap=[[pstr(PW0S), 11], [100, 8]])
                dst_m1 = bass.AP(tensor=T1.tensor,
                                 offset=T1.offset + 75 * pstr(T1) + 760 + n * 8,
                                 ap=[[pstr(T1), 11], [1, 8]])
                nc.sync.dma_start(out=dst_m1.bitcast(F32R),
                                  in_=src_m1.bitcast(F32R))

            # T3 per-j: dst [j*8+o rows, col 115g+j], src PW23S[o, 15g+j]
            dma_rot3 = [nc.sync, nc.gpsimd]
            for j in range(15):
                gcnt = 7 if j < 10 else 6
                s = bass.AP(tensor=PW23S.tensor, offset=PW23S.offset + j,
                            ap=[[pstr(PW23S), 8], [15, gcnt], [1, 1]])
                dst = bass.AP(tensor=T3.tensor,
                              offset=T3.offset + j * 8 * pstr(T3) + j,
                              ap=[[pstr(T3), 8], [115, gcnt], [1, 1]])
                dma_rot3[j % 2].dma_start(out=dst.bitcast(F32R),
                                          in_=s.bitcast(F32R))
            # T3 b2 row: dst [120, 115g+j], padded to 7 groups x 15
            srcb2a = bass.AP(tensor=PW23S.tensor,
                             offset=PW23S.offset + 8 * pstr(PW23S),
                             ap=[[pstr(PW23S), 1], [15, 7], [1, 15]])
            dstb2a = bass.AP(tensor=T3.tensor, offset=T3.offset + 120 * pstr(T3),
                             ap=[[pstr(T3), 1], [115, 7], [1, 15]])
            nc.gpsimd.dma_start(out=dstb2a.bitcast(F32R), in_=srcb2a.bitcast(F32R))

            # ---------------- main loop (skewed) ----------------
            psm_cm = tc.tile_pool(name="psmain", bufs=1, space="PSUM")
            psm = psm_cm.__enter__()
            ps1 = [psm.tile([120, CHUNK], F32, name=f"ps1_{i}", tag=f"ps1_{i}")
                   for i in range(3)]
            ps2 = [psm.tile([120, CHUNK], F32, name=f"ps2_{i}", tag=f"ps2_{i}")
                   for i in range(3)]
            ps3 = [psm.tile([NQ, CHUNK], F32, name=f"ps3_{i}", tag=f"ps3_{i}")
                   for i in range(2)]

            out_q = [nc.sync, nc.gpsimd]

            def emit_L3(cp):
                coff, clen = CHUNKS[cp]
                ph = ps3[cp % 2]
                for g in range(7):
                    nc.tensor.matmul(
                        ph[0:NQ, 0:clen],
                        T3[0:121, 100 * g:100 * g + NQ],
                        y2slice(g, cp % 5, clen),
                        start=(g == 0), stop=(g == 6),
                        skip_group_check=True)

            def emit_L3copy(cp):
                coff, clen = CHUNKS[cp]
                ph = ps3[cp % 2]
                qi = next(i for i, (a, b) in enumerate(QUARTERS) if a <= cp < b)
                q0 = CHUNKS[QUARTERS[qi][0]][0]
                stg = STG[qi % 2]
                nc.scalar.activation(stg[0:NQ, coff - q0:coff - q0 + clen],
                                     ph[0:NQ, 0:clen], Copy)
                if cp == QUARTERS[qi][1] - 1:
                    qlen = coff + clen - q0
                    last = (qi == len(QUARTERS) - 1)
                    qa = out_q[qi % 2]
                    qb = out_q[(qi + 1) % 2]
                    qc = nc.scalar if last else out_q[qi % 2]
                    qa.dma_start(out=outp[0:45, q0:q0 + qlen],
                                 in_=stg[0:45, 0:qlen])
                    qb.dma_start(out=outp[50:95, q0:q0 + qlen],
                                 in_=stg[45:90, 0:qlen])
                    qc.dma_start(out=outp[45:50, q0:q0 + qlen],
                                 in_=stg[90:95, 0:qlen])
                    qc.dma_start(out=outp[95:100, q0:q0 + qlen],
                                 in_=stg[95:100, 0:qlen])

            def emit_L1(ci, g):
                coff, clen = CHUNKS[ci]
                band, k1, nq = GROUPS[g]
                m = nq * 8
                pa = ps1[(ci * 7 + g) % 3]
                nc.tensor.matmul(
                    pa[0:m, 0:clen],
                    T1[band:band + k1, g * 120:g * 120 + m],
                    Ft[band:band + k1, coff:coff + clen],
                    start=True, stop=True)
                relu = relu_act if g % 2 == 0 else relu_dve
                relu(y1slice(g, ci % 2, clen, rows=m), pa[0:m, 0:clen])

            def emit_L2(ci, g):
                coff, clen = CHUNKS[ci]
                _, _, nq = GROUPS[g]
                m = nq * 8
                pb = ps2[(ci * 7 + g) % 3]
                nc.tensor.matmul(
                    pb[0:m, 0:clen],
                    T2[0:121, g * 120:g * 120 + m],
                    y1slice(g, ci % 2, clen),
                    start=True, stop=True)
                relu = relu_dve if g % 2 == 0 else relu_act
                relu(y2slice(g, ci % 5, clen, rows=m), pb[0:m, 0:clen])

            l3_next = [0]
            for ci in range(NCH + 2):
                pend = []
                if ci < NCH:
                    for g in range(3):
                        emit_L1(ci, g)
                if ci >= 4:
                    # catch-up: 2 chains while behind, and again at the tail
                    n_emit = 2 if (l3_next[0] < ci - 3 or ci >= NCH - 1) else 1
                    limit = ci - 1 if ci >= NCH - 1 else ci - 2
                    for _ in range(n_emit):
                        if l3_next[0] <= min(limit, NCH - 1):
                            emit_L3(l3_next[0])
                            pend.append(l3_next[0])
                            l3_next[0] += 1
                if ci < NCH:
                    for g in range(7):
                        emit_L2(ci, g)
                        if g + 3 < 7:
                            emit_L1(ci, g + 3)
                        if g == 0:
                            for cp in pend:
                                emit_L3copy(cp)
                            pend = []
                for cp in pend:
                    emit_L3copy(cp)
            while l3_next[0] < NCH:
                emit_L3(l3_next[0])
                emit_L3copy(l3_next[0])
                l3_next[0] += 1
            psm_cm.__exit__(None, None, None)

    nc.compile()
    return nc


_NC = None


def _get_nc():
    global _NC
    if _NC is None:
        _NC = _build_program()
    return _NC


# n' permutation: group-major query order per core
NPERM = np.concatenate([np.arange(0, 45), np.arange(50, 95),
                        np.arange(45, 50), np.arange(95, 100)])


def _host_pack(hs, mask_features, references, sizes, W_ctrl, b_ctrl):
    hs = np.asarray(hs, np.float32)
    mask_features = np.asarray(mask_features, np.float32)
    references = np.asarray(references, np.float32)
    sizes = np.asarray(sizes, np.float32)
    W_ctrl = np.asarray(W_ctrl, np.float32)
    b_ctrl = np.asarray(b_ctrl, np.float32)

    # pixel grid
    xs = np.arange(W, dtype=np.float32) * STRIDE + STRIDE // 2
    ys = np.arange(H, dtype=np.float32) * STRIDE + STRIDE // 2
    gxf = np.tile(xs, H)
    gyf = np.repeat(ys, W)
    cstm = np.stack([-gxf, -gyf, np.ones(P, np.float32)]).astype(np.float32)

    # wall [771, 169]: rows contract against [hs; hs*rx; hs*ry; 1; rx; ry]
    W_aug = np.concatenate([W_ctrl.T, b_ctrl[None, :]], 0)  # [257, 169]
    wall = np.zeros((771, 169), np.float32)

    def setcol(c, p):
        wall[0:256, c] = W_aug[0:256, p]
        wall[768, c] = W_aug[256, p]

    for o in range(8):
        for i in range(10):
            setcol(o * 11 + i, o * 10 + i)
        c = o * 11 + 10                      # c = w8*rx + w9*ry + b0
        wall[256:512, c] = W_aug[0:256, o * 10 + 8]
        wall[769, c] = W_aug[256, o * 10 + 8]
        wall[512:768, c] = W_aug[0:256, o * 10 + 9]
        wall[770, c] = W_aug[256, o * 10 + 9]
        wall[0:256, c] = W_aug[0:256, 152 + o]
        wall[768, c] = W_aug[256, 152 + o]
    for o2 in range(8):
        for j in range(8):
            setcol(88 + o2 * 9 + j, 80 + o2 * 8 + j)
        setcol(88 + o2 * 9 + 8, 160 + o2)
    for j in range(8):
        setcol(160 + j, 144 + j)
    setcol(168, 168)
    wpk = np.zeros((128, 1183), np.float32)
    for s, kc in enumerate([0, 1, None, 2, 3, 4, 5]):
        if kc is None:
            wpk[0:3, s * 169:s * 169 + 169] = wall[768:771]
        else:
            wpk[:, s * 169:s * 169 + 169] = wall[128 * kc:128 * (kc + 1)]
    wall = wpk.astype(np.float16)

    # reference points in pixels
    b_idx = np.arange(16) // 8
    scale = sizes[b_idx][:, ::-1]                      # [16, 2] = (img_w, img_h)
    refs_px = references * scale[:, None, :]           # [16, 50, 2]

    in_maps = []
    for c in range(NCORES):
        hs_c = hs[2 * c:2 * c + 2].reshape(NQ, HID)[NPERM]
        mf_c = mask_features[2 * c:2 * c + 2].reshape(2, 8, P)
        rp = refs_px[2 * c:2 * c + 2].reshape(NQ, 2)[NPERM]
        hrows = np.empty((771, NQ), np.float32)
        hrows[0:256] = hs_c.T
        hrows[256:512] = hs_c.T * rp[:, 0][None, :]
        hrows[512:768] = hs_c.T * rp[:, 1][None, :]
        hrows[768] = 1.0
        hrows[769] = rp[:, 0]
        hrows[770] = rp[:, 1]
        hpk = np.zeros((128, 700), np.float32)
        for s, kc in enumerate([0, 1, None, 2, 3, 4, 5]):
            if kc is None:
                hpk[0:3, s * 100:s * 100 + 100] = hrows[768:771]
            else:
                hpk[:, s * 100:s * 100 + 100] = hrows[128 * kc:128 * (kc + 1)]
        in_maps.append(dict(
            mf=np.ascontiguousarray(mf_c),
            hsall=hpk.astype(np.float16),
            wall=wall,
            cst=cstm,
        ))
    return in_maps


def kernel(hs, mask_features, references, sizes, W_ctrl, b_ctrl, T):
    assert int(T) == 8
    nc = _get_nc()
    in_maps = _host_pack(hs, mask_features, references, sizes, W_ctrl, b_ctrl)
    res = bass_utils.run_bass_kernel_spmd(nc, in_maps, core_ids=list(range(NCORES)))
    out = np.empty((16, Q, H, W), np.float32)
    for c in range(NCORES):
        out[2 * c:2 * c + 2] = res.results[c]["outp"].reshape(2, Q, H, W)
    return out


# revision 8
# speedup vs baseline: 1.0001x; 1.0001x over previous
"""DynamicSegmentationHead Trainium2 kernel, restructured for overlap.

Data-parallel over 16 clip-frames: each core handles 2 frames (100 queries).
Query order is host-permuted to group-major n' = [f0 q0-44, f1 q0-44,
f0 q45-49, f1 q45-49] so every weight-scatter is one uniform strided DMA
and the L3 psum block maps to output rows with 3 clean DMAs per flush.

Controller runs in fp16 over an extended K of 771: host supplies
[hs; hs*refx; hs*refy; ones; refx; refy] so the per-query constant row
c = w8*refx + w9*refy + b0 falls out of the matmul directly (no separate
c-row pipeline). pw23 -> pw1 -> pw0 ordering releases the T3/T2/T1 build
paths in deadline order.

Per core, per 512-pixel chunk (17 chunks):
  L1: y1 = relu(T1 @ Ft)    7 matmuls K=11/22, M=120/80, psum rot x3
  L2: y2 = relu(T2 @ y1)    7 matmuls K=121 blockdiag, psum rot x3
  L3: one [100,512] psum, 7 accumulating matmuls, emitted with a
      progressive 2-chunk skew (first chains deferred to iteration 4 so
      they never wait on the T3 scatter DMAs), Y2 5-deep
  ACT stage copy + output flush DMAs per span, relus split ACT/DVE.

Steady state is ACT/DVE-bound: 15 psum-evacuation ops per chunk across
2 engines is the structural floor (PSUM has 8 banks; DMA cannot read
PSUM; 7 query groups is minimal since 800 M-rows / 128 partitions).
"""

import numpy as np

import concourse.bass as bass
import concourse.bacc as bacc
import concourse.tile as tile
from concourse import mybir
from concourse import bass_utils

F32 = mybir.dt.float32
F32R = mybir.dt.float32r
F16 = mybir.dt.float16

HID = 256
Q = 50
H, W = 72, 120
P = H * W            # 8640
NQ = 100             # queries per core (2 frames)
NCORES = 8
STRIDE = 4

CHUNK = 512
NCH = 17
CHUNKS = [(i * CHUNK, CHUNK) for i in range(16)] + [(16 * CHUNK, P - 16 * CHUNK)]
QUARTERS = [(0, 4), (4, 8), (8, 12), (12, 16), (16, 17)]   # flush spans

# groups in n' (group-major) order: 3 f0, 3 f1, 1 mixed
# (band, k1, nq); n'-base of group g is 15*g
GROUPS = [(0, 11, 15)] * 3 + [(32, 11, 15)] * 3 + [(64, 22, 10)]

Relu = mybir.ActivationFunctionType.Relu
Copy = mybir.ActivationFunctionType.Copy


def _build_program():
    nc = bacc.Bacc("TRN2", target_bir_lowering=False, debug=False)
    R = lambda ap: ap.bitcast(F32R)
    FV = lambda ap: ap.bitcast(F32)

    mf = nc.dram_tensor("mf", [2, 8, P], F32, kind="ExternalInput").ap()
    # hsall: 7 k-chunks of [hs(256); hs*rx(256); hs*ry(256); 1; rx; ry],
    # packed column-major in chunk order [0, 1, 6, 2, 3, 4, 5] so the
    # L2/L3-critical chunks (0, 1, 6) transfer first.
    hsall = nc.dram_tensor("hsall", [128, 700], F16, kind="ExternalInput").ap()
    # wall: [3 x 81 w23+w1 cols (slots 0-2)] then [7 x 88 w0 cols]
    wall = nc.dram_tensor("wall", [128, 859], F16, kind="ExternalInput").ap()
    cst = nc.dram_tensor("cst", [3, P], F32, kind="ExternalInput").ap()
    outp = nc.dram_tensor("outp", [NQ, P], F32, kind="ExternalOutput").ap()

    def relu_act(out_ap, in_ap):
        nc.scalar.activation(out_ap, in_ap, Relu)

    def relu_dve(out_ap, in_ap):
        nc.vector.tensor_scalar(out=out_ap, in0=in_ap, scalar1=0.0,
                                scalar2=None, op0=mybir.AluOpType.max)

    with tile.TileContext(nc) as tc:
        with tc.tile_pool(name="persist", bufs=1) as pers:
            # ---------------- persistent SBUF ----------------
            Ft = pers.tile([128, P], F32R, tag="F")
            HSB = pers.tile([128, 700], F16, tag="HSB")
            WSB = pers.tile([128, 859], F16, tag="WSB")
            PW0S = pers.tile([11, 800], F32, tag="PW0S")
            PW1S = pers.tile([9, 840], F32, tag="PW1S")
            PW23S = pers.tile([9, 105], F32, tag="PW23S")

            T1 = pers.tile([86, 840], F32R, tag="T1")
            T2 = pers.tile([121, 840], F32R, tag="T2")
            T3 = pers.tile([121, 705], F32R, tag="T3")
            Y1 = pers.tile([121, 7168], F32R, tag="Y1")   # (g,buf) col blocks
            Y2 = pers.tile([121, 17920], F32R, tag="Y2")
            STG = [pers.tile([NQ, 2048], F32, name=f"STG{i}", tag=f"STG{i}")
                   for i in range(2)]

            pstr = lambda t: t.ap[0][0]   # partition stride (elements)

            def y1slice(g, buf, cl, rows=121):
                base = (2 * g + buf) * CHUNK
                return Y1[0:rows, base:base + cl]

            def y2slice(g, buf, cl, rows=121):
                base = (5 * g + buf) * CHUNK
                return Y2[0:rows, base:base + cl]

            # ---------------- input DMAs ----------------
            nc.sync.dma_start(out=HSB[:, 0:300], in_=hsall[:, 0:300])
            nc.scalar.dma_start(out=WSB[:, 0:243], in_=wall[:, 0:243])
            nc.sync.dma_start(out=HSB[:, 300:700], in_=hsall[:, 300:700])
            nc.scalar.dma_start(out=WSB[:, 243:859], in_=wall[:, 243:859])

            # F layout: [0:8]=feat0, [8:10]=-gx,-gy, [10]=ones,
            #           [32:40]=feat1, [40:42]=-gx,-gy, [42]=ones,
            #           [64:72]=feat0, [72:74]=-gx,-gy, [74]=ones,
            #           [75:83]=feat1, [83:85]=-gx,-gy, [85]=ones
            nc.sync.dma_start(out=R(Ft[0:8, :]), in_=R(mf[0]))
            nc.sync.dma_start(out=R(Ft[8:11, :]), in_=R(cst[0:3, :]))
            nc.scalar.dma_start(out=R(Ft[32:40, :]), in_=R(mf[1]))
            nc.scalar.dma_start(out=R(Ft[40:43, :]), in_=R(cst[0:3, :]))
            # mixed band: SBUF->SBUF copies of the two main bands
            with tc.high_priority():
                nc.gpsimd.dma_start(out=R(Ft[64:75, :]), in_=R(Ft[0:11, :]))
                nc.gpsimd.dma_start(out=R(Ft[75:86, :]), in_=R(Ft[32:43, :]))
                # ones rows of Y1/Y2 (row 120) from the Ft ones row
                nc.gpsimd.dma_start(out=FV(Y1[120:121, :]),
                                    in_=FV(Ft[10:11, 0:7168]))
                nc.gpsimd.dma_start(out=FV(Y2[120:121, 0:8640]),
                                    in_=FV(Ft[10:11, 0:8640]))
                nc.gpsimd.dma_start(out=FV(Y2[120:121, 8640:17280]),
                                    in_=FV(Ft[10:11, 0:8640]))
                nc.gpsimd.dma_start(out=FV(Y2[120:121, 17280:17920]),
                                    in_=FV(Ft[10:11, 0:640]))

            # ---------------- zero fills ----------------
            nc.gpsimd.memset(FV(T2[:, :]), 0.0)
            nc.gpsimd.memset(FV(T3[:, :]), 0.0)
            nc.gpsimd.memset(FV(T1[64:86, 720:840]), 0.0)
            nc.gpsimd.memset(PW1S[0:9, 800:840], 0.0)
            nc.gpsimd.memset(PW23S[0:9, 100:105], 0.0)
            # mixed-group stale rows (80-119) of Y1/Y2 col blocks
            nc.gpsimd.memset(FV(Y1[64:120, 6144:7168]), 0.0)
            nc.gpsimd.memset(FV(Y2[64:120, 15360:17920]), 0.0)

            # ---------------- controller matmuls (fp16) ----------------
            with tc.tile_pool(name="psctrl", bufs=1, space="PSUM") as psc:
                pw1p = psc.tile([9, 1024], F32, tag="pw1p")
                pw23p = psc.tile([9, NQ], F32, tag="pw23p")
                pw0p = psc.tile([11, 1024], F32, tag="pw0p")
                # packed slot s holds logical chunk [0,1,6,2,3,4,5][s]
                # L2/L3 params contract over hs + tail chunks (slots 0,1,2)
                k3 = [(0, 128), (1, 128), (2, 3)]
                # L1 params (incl c row) contract over all 7 slots
                k7 = [(0, 128), (1, 128), (3, 128), (4, 128), (5, 128),
                      (6, 128), (2, 3)]
                # w1/b1 params first: T2 build is the long pole
                for i, (kc, kn) in enumerate(k3):
                    c0 = kc * 81
                    nc.tensor.matmul(
                        pw23p[0:9, 0:NQ],
                        WSB[0:kn, c0:c0 + 9],
                        HSB[0:kn, kc * 100:kc * 100 + NQ],
                        start=(i == 0), stop=(i == 2))
                for o2 in range(8):
                    for i, (kc, kn) in enumerate(k3):
                        c0 = kc * 81 + 9 + o2 * 9
                        nc.tensor.matmul(
                            pw1p[0:9, o2 * 128:o2 * 128 + NQ],
                            WSB[0:kn, c0:c0 + 9],
                            HSB[0:kn, kc * 100:kc * 100 + NQ],
                            start=(i == 0), stop=(i == 2))
                for o in range(8):
                    for i, (kc, kn) in enumerate(k7):
                        c0 = 243 + kc * 88 + o * 11
                        nc.tensor.matmul(
                            pw0p[0:11, o * 128:o * 128 + NQ],
                            WSB[0:kn, c0:c0 + 11],
                            HSB[0:kn, kc * 100:kc * 100 + NQ],
                            start=(i == 0), stop=(i == 6))

                # evacuate psum -> SBUF (the only way out of PSUM)
                nc.vector.tensor_copy(
                    PW1S[:, 0:800].rearrange("p (n o) -> p o n", o=8),
                    pw1p.rearrange("p (o n) -> p o n", o=8)[:, :, 0:NQ])
                nc.scalar.activation(PW23S[:, 0:NQ], pw23p[:, :], Copy)
                nc.scalar.activation(
                    PW0S.rearrange("p (o n) -> p o n", o=8),
                    pw0p.rearrange("p (o n) -> p o n", o=8)[:, :, 0:NQ], Copy)

            # ---------------- weight scatters ----------------
            # T2 per-j: dst [j*8:(j+1)*8, g*120+j*8+o2], src PW1S[o, (15g+j)*8+o2]
            dma_rot = [nc.sync, nc.sync, nc.sync, nc.gpsimd]
            for j in range(15):
                gcnt = 7 if j < 10 else 6
                if j % 4 == 0:
                    # 32-aligned partition base: engine copy
                    dst = T2[j * 8:j * 8 + 8, :].rearrange(
                        "p (g x) -> p g x", g=7)[:, 0:gcnt, j * 8:j * 8 + 8]
                    sv = PW1S[0:8, 0:840].rearrange(
                        "p (g x) -> p g x", g=7)[:, 0:gcnt, j * 8:j * 8 + 8]
                    if j % 8 == 0:
                        nc.vector.tensor_copy(dst, sv)
                    else:
                        nc.scalar.activation(dst, sv, Copy)
                else:
                    s = bass.AP(tensor=PW1S.tensor, offset=PW1S.offset + j * 8,
                                ap=[[pstr(PW1S), 8], [120, gcnt], [1, 8]])
                    dst = bass.AP(tensor=T2.tensor,
                                  offset=T2.offset + j * 8 * pstr(T2) + j * 8,
                                  ap=[[pstr(T2), 8], [120, gcnt], [1, 8]])
                    dma_rot[j % 4].dma_start(out=dst.bitcast(F32R),
                                             in_=s.bitcast(F32R))
            # T2 b1 row: dst [120, g*120+j*8+o2] <- PW1S[8, (15g+j)*8+o2]
            srcb1 = bass.AP(tensor=PW1S.tensor, offset=PW1S.offset + 8 * pstr(PW1S),
                            ap=[[pstr(PW1S), 1], [120, 7], [1, 120]])
            dstb1 = bass.AP(tensor=T2.tensor, offset=T2.offset + 120 * pstr(T2),
                            ap=[[pstr(T2), 1], [120, 7], [1, 120]])
            nc.sync.dma_start(out=dstb1.bitcast(F32R), in_=srcb1.bitcast(F32R))

            # T1 scatters (engine copies where partition base allows)
            # f0 block: rows 0-9 <- PW0S rows 0-9, cols (g,j,o) g-major
            src_f0 = bass.AP(tensor=PW0S.tensor, offset=PW0S.offset,
                             ap=[[pstr(PW0S), 11], [1, 45], [100, 8]])
            nc.vector.tensor_copy(T1[0:11, 0:360].rearrange(
                "p (n o) -> p n o", o=8), src_f0)
            src_f1 = bass.AP(tensor=PW0S.tensor, offset=PW0S.offset + 45,
                             ap=[[pstr(PW0S), 11], [1, 45], [100, 8]])
            nc.scalar.activation(T1[32:43, 360:720].rearrange(
                "p (n o) -> p n o", o=8), src_f1, Copy)
            # mixed f0-half rows 64-74 incl c (n' 90-94), base 64 ok
            src_m0 = bass.AP(tensor=PW0S.tensor, offset=PW0S.offset + 90,
                             ap=[[pstr(PW0S), 11], [1, 5], [100, 8]])
            nc.vector.tensor_copy(T1[64:75, 720:760].rearrange(
                "p (n o) -> p n o", o=8), src_m0)
            # mixed f1-half rows 75-85 incl c (n' 95-99): base 75 -> DMA/query
            for n in range(5):
                src_m1 = bass.AP(tensor=PW0S.tensor, offset=PW0S.offset + 95 + n,
                                 ap=[[pstr(PW0S), 11], [100, 8]])
                dst_m1 = bass.AP(tensor=T1.tensor,
                                 offset=T1.offset + 75 * pstr(T1) + 760 + n * 8,
                                 ap=[[pstr(T1), 11], [1, 8]])
                nc.sync.dma_start(out=dst_m1.bitcast(F32R),
                                  in_=src_m1.bitcast(F32R))

            # T3 per-j: dst [j*8+o rows, col 115g+j], src PW23S[o, 15g+j]
            dma_rot3 = [nc.sync, nc.gpsimd]
            for j in range(15):
                gcnt = 7 if j < 10 else 6
                s = bass.AP(tensor=PW23S.tensor, offset=PW23S.offset + j,
                            ap=[[pstr(PW23S), 8], [15, gcnt], [1, 1]])
                dst = bass.AP(tensor=T3.tensor,
                              offset=T3.offset + j * 8 * pstr(T3) + j,
                              ap=[[pstr(T3), 8], [115, gcnt], [1, 1]])
                dma_rot3[j % 2].dma_start(out=dst.bitcast(F32R),
                                          in_=s.bitcast(F32R))
            # T3 b2 row: dst [120, 115g+j], padded to 7 groups x 15
            srcb2a = bass.AP(tensor=PW23S.tensor,
                             offset=PW23S.offset + 8 * pstr(PW23S),
                             ap=[[pstr(PW23S), 1], [15, 7], [1, 15]])
            dstb2a = bass.AP(tensor=T3.tensor, offset=T3.offset + 120 * pstr(T3),
                             ap=[[pstr(T3), 1], [115, 7], [1, 15]])
            nc.gpsimd.dma_start(out=dstb2a.bitcast(F32R), in_=srcb2a.bitcast(F32R))

            # ---------------- main loop (skewed) ----------------
            psm_cm = tc.tile_pool(name="psmain", bufs=1, space="PSUM")
            psm = psm_cm.__enter__()
            ps1 = [psm.tile([120, CHUNK], F32, name=f"ps1_{i}", tag=f"ps1_{i}")
                   for i in range(3)]
            ps2 = [psm.tile([120, CHUNK], F32, name=f"ps2_{i}", tag=f"ps2_{i}")
                   for i in range(3)]
            ps3 = [psm.tile([NQ, CHUNK], F32, name=f"ps3_{i}", tag=f"ps3_{i}")
                   for i in range(2)]

            out_q = [nc.sync, nc.gpsimd]

            def emit_L3(cp):
                coff, clen = CHUNKS[cp]
                ph = ps3[cp % 2]
                for g in range(7):
                    nc.tensor.matmul(
                        ph[0:NQ, 0:clen],
                        T3[0:121, 100 * g:100 * g + NQ],
                        y2slice(g, cp % 5, clen),
                        start=(g == 0), stop=(g == 6),
                        skip_group_check=True)

            def emit_L3copy(cp):
                coff, clen = CHUNKS[cp]
                ph = ps3[cp % 2]
                qi = next(i for i, (a, b) in enumerate(QUARTERS) if a <= cp < b)
                q0 = CHUNKS[QUARTERS[qi][0]][0]
                stg = STG[qi % 2]
                nc.scalar.activation(stg[0:NQ, coff - q0:coff - q0 + clen],
                                     ph[0:NQ, 0:clen], Copy)
                if cp == QUARTERS[qi][1] - 1:
                    qlen = coff + clen - q0
                    last = (qi == len(QUARTERS) - 1)
                    qa = out_q[qi % 2]
                    qb = out_q[(qi + 1) % 2]
                    qc = nc.scalar if last else out_q[qi % 2]
                    qa.dma_start(out=outp[0:45, q0:q0 + qlen],
                                 in_=stg[0:45, 0:qlen])
                    qb.dma_start(out=outp[50:95, q0:q0 + qlen],
                                 in_=stg[45:90, 0:qlen])
                    qc.dma_start(out=outp[45:50, q0:q0 + qlen],
                                 in_=stg[90:95, 0:qlen])
                    qc.dma_start(out=outp[95:100, q0:q0 + qlen],
                                 in_=stg[95:100, 0:qlen])

            def emit_L1(ci, g):
                coff, clen = CHUNKS[ci]
                band, k1, nq = GROUPS[g]
                m = nq * 8
                pa = ps1[(ci * 7 + g) % 3]
                nc.tensor.matmul(
                    pa[0:m, 0:clen],
                    T1[band:band + k1, g * 120:g * 120 + m],
                    Ft[band:band + k1, coff:coff + clen],
                    start=True, stop=True)
                relu = relu_act if g % 2 == 0 else relu_dve
                relu(y1slice(g, ci % 2, clen, rows=m), pa[0:m, 0:clen])

            def emit_L2(ci, g):
                coff, clen = CHUNKS[ci]
                _, _, nq = GROUPS[g]
                m = nq * 8
                pb = ps2[(ci * 7 + g) % 3]
                nc.tensor.matmul(
                    pb[0:m, 0:clen],
                    T2[0:121, g * 120:g * 120 + m],
                    y1slice(g, ci % 2, clen),
                    start=True, stop=True)
                relu = relu_dve if g % 2 == 0 else relu_act
                relu(y2slice(g, ci % 5, clen, rows=m), pb[0:m, 0:clen])

            l3_next = [0]
            for ci in range(NCH + 2):
                pend = []
                if ci < NCH:
                    for g in range(3):
                        emit_L1(ci, g)
                if ci >= 4:
                    # catch-up: 2 chains while behind, and again at the tail
                    n_emit = 2 if (l3_next[0] < ci - 3 or ci >= NCH - 1) else 1
                    limit = ci - 1 if ci >= NCH - 1 else ci - 2
                    for _ in range(n_emit):
                        if l3_next[0] <= min(limit, NCH - 1):
                            emit_L3(l3_next[0])
                            pend.append(l3_next[0])
                            l3_next[0] += 1
                if ci < NCH:
                    for g in range(7):
                        emit_L2(ci, g)
                        if g + 3 < 7:
                            emit_L1(ci, g + 3)
                        if g == 0:
                            for cp in pend:
                                emit_L3copy(cp)
                            pend = []
                for cp in pend:
                    emit_L3copy(cp)
            while l3_next[0] < NCH:
                emit_L3(l3_next[0])
                emit_L3copy(l3_next[0])
                l3_next[0] += 1
            psm_cm.__exit__(None, None, None)

    nc.compile()
    return nc


_NC = None


def _get_nc():
    global _NC
    if _NC is None:
        _NC = _build_program()
    return _NC


# n' permutation: group-major query order per core
NPERM = np.concatenate([np.arange(0, 45), np.arange(50, 95),
                        np.arange(45, 50), np.arange(95, 100)])


def _host_pack(hs, mask_features, references, sizes, W_ctrl, b_ctrl):
    hs = np.asarray(hs, np.float32)
    mask_features = np.asarray(mask_features, np.float32)
    references = np.asarray(references, np.float32)
    sizes = np.asarray(sizes, np.float32)
    W_ctrl = np.asarray(W_ctrl, np.float32)
    b_ctrl = np.asarray(b_ctrl, np.float32)

    # pixel grid
    xs = np.arange(W, dtype=np.float32) * STRIDE + STRIDE // 2
    ys = np.arange(H, dtype=np.float32) * STRIDE + STRIDE // 2
    gxf = np.tile(xs, H)
    gyf = np.repeat(ys, W)
    cstm = np.stack([-gxf, -gyf, np.ones(P, np.float32)]).astype(np.float32)

    # wall [771, 169]: rows contract against [hs; hs*rx; hs*ry; 1; rx; ry]
    W_aug = np.concatenate([W_ctrl.T, b_ctrl[None, :]], 0)  # [257, 169]
    wall = np.zeros((771, 169), np.float32)

    def setcol(c, p):
        wall[0:256, c] = W_aug[0:256, p]
        wall[768, c] = W_aug[256, p]

    for o in range(8):
        for i in range(10):
            setcol(o * 11 + i, o * 10 + i)
        c = o * 11 + 10                      # c = w8*rx + w9*ry + b0
        wall[256:512, c] = W_aug[0:256, o * 10 + 8]
        wall[769, c] = W_aug[256, o * 10 + 8]
        wall[512:768, c] = W_aug[0:256, o * 10 + 9]
        wall[770, c] = W_aug[256, o * 10 + 9]
        wall[0:256, c] = W_aug[0:256, 152 + o]
        wall[768, c] = W_aug[256, 152 + o]
    for o2 in range(8):
        for j in range(8):
            setcol(88 + o2 * 9 + j, 80 + o2 * 8 + j)
        setcol(88 + o2 * 9 + 8, 160 + o2)
    for j in range(8):
        setcol(160 + j, 144 + j)
    setcol(168, 168)
    # packed slot s holds logical k-chunk [0, 1, 6(tail), 2, 3, 4, 5][s]:
    # first 3x81 = w2/b2 + w1/b1 cols of slots 0-2, then 7x88 w0 cols
    wpk = np.zeros((128, 859), np.float32)
    for s, kc in enumerate([0, 1, None, 2, 3, 4, 5]):
        rows = wall[768:771] if kc is None else wall[128 * kc:128 * (kc + 1)]
        nr = rows.shape[0]
        if s < 3:
            wpk[0:nr, s * 81:s * 81 + 9] = rows[:, 160:169]
            wpk[0:nr, s * 81 + 9:s * 81 + 81] = rows[:, 88:160]
        wpk[0:nr, 243 + s * 88:243 + s * 88 + 88] = rows[:, 0:88]
    wall = wpk.astype(np.float16)

    # reference points in pixels
    b_idx = np.arange(16) // 8
    scale = sizes[b_idx][:, ::-1]                      # [16, 2] = (img_w, img_h)
    refs_px = references * scale[:, None, :]           # [16, 50, 2]

    in_maps = []
    for c in range(NCORES):
        hs_c = hs[2 * c:2 * c + 2].reshape(NQ, HID)[NPERM]
        mf_c = mask_features[2 * c:2 * c + 2].reshape(2, 8, P)
        rp = refs_px[2 * c:2 * c + 2].reshape(NQ, 2)[NPERM]
        hrows = np.empty((771, NQ), np.float32)
        hrows[0:256] = hs_c.T
        hrows[256:512] = hs_c.T * rp[:, 0][None, :]
        hrows[512:768] = hs_c.T * rp[:, 1][None, :]
        hrows[768] = 1.0
        hrows[769] = rp[:, 0]
        hrows[770] = rp[:, 1]
        hpk = np.zeros((128, 700), np.float32)
        for s, kc in enumerate([0, 1, None, 2, 3, 4, 5]):
            if kc is None:
                hpk[0:3, s * 100:s * 100 + 100] = hrows[768:771]
            else:
                hpk[:, s * 100:s * 100 + 100] = hrows[128 * kc:128 * (kc + 1)]
        in_maps.append(dict(
            mf=np.ascontiguousarray(mf_c),
            hsall=hpk.astype(np.float16),
            wall=wall,
            cst=cstm,
        ))
    return in_maps


def kernel(hs, mask_features, references, sizes, W_ctrl, b_ctrl, T):
    assert int(T) == 8
    nc = _get_nc()
    in_maps = _host_pack(hs, mask_features, references, sizes, W_ctrl, b_ctrl)
    res = bass_utils.run_bass_kernel_spmd(nc, in_maps, core_ids=list(range(NCORES)))
    out = np.empty((16, Q, H, W), np.float32)
    for c in range(NCORES):
        out[2 * c:2 * c + 2] = res.results[c]["outp"].reshape(2, Q, H, W)
    return out


# revision 9
# speedup vs baseline: 1.0204x; 1.0203x over previous
"""DynamicSegmentationHead Trainium2 kernel, restructured for overlap.

Data-parallel over 16 clip-frames: each core handles 2 frames (100 queries).
Query order is host-permuted to group-major n' = [f0 q0-44, f1 q0-44,
f0 q45-49, f1 q45-49] so every weight-scatter is one uniform strided DMA
and the L3 psum block maps to output rows with 3 clean DMAs per flush.

Controller runs in fp16 over an extended K of 771: host supplies
[hs; hs*refx; hs*refy; ones; refx; refy] so the per-query constant row
c = w8*refx + w9*refy + b0 falls out of the matmul directly (no separate
c-row pipeline). pw23 -> pw1 -> pw0 ordering releases the T3/T2/T1 build
paths in deadline order.

Per core, per 512-pixel chunk (17 chunks):
  L1: y1 = relu(T1 @ Ft)    7 matmuls K=11/22, M=120/80, psum rot x3
  L2: y2 = relu(T2 @ y1)    7 matmuls K=121 blockdiag, psum rot x3
  L3: one [100,512] psum, 7 accumulating matmuls, emitted with a
      progressive 2-chunk skew (first chains deferred to iteration 4 so
      they never wait on the T3 scatter DMAs), Y2 5-deep
  ACT stage copy + output flush DMAs per span, relus split ACT/DVE.

Steady state is ACT/DVE-bound: 15 psum-evacuation ops per chunk across
2 engines is the structural floor (PSUM has 8 banks; DMA cannot read
PSUM; 7 query groups is minimal since 800 M-rows / 128 partitions).
"""

import numpy as np

import concourse.bass as bass
import concourse.bacc as bacc
import concourse.tile as tile
from concourse import mybir
from concourse import bass_utils

F32 = mybir.dt.float32
F32R = mybir.dt.float32r
F16 = mybir.dt.float16

HID = 256
Q = 50
H, W = 72, 120
P = H * W            # 8640
NQ = 100             # queries per core (2 frames)
NCORES = 8
STRIDE = 4

CHUNK = 512
NCH = 17
CHUNKS = [(i * CHUNK, CHUNK) for i in range(16)] + [(16 * CHUNK, P - 16 * CHUNK)]
QUARTERS = [(0, 4), (4, 8), (8, 12), (12, 16), (16, 17)]   # flush spans

# groups in n' (group-major) order: 3 f0, 3 f1, 1 mixed
# (band, k1, nq); n'-base of group g is 15*g
GROUPS = [(0, 11, 15)] * 3 + [(32, 11, 15)] * 3 + [(64, 22, 10)]

Relu = mybir.ActivationFunctionType.Relu
Copy = mybir.ActivationFunctionType.Copy


def _build_program():
    nc = bacc.Bacc("TRN2", target_bir_lowering=False, debug=False)
    R = lambda ap: ap.bitcast(F32R)
    FV = lambda ap: ap.bitcast(F32)

    mf = nc.dram_tensor("mf", [2, 8, P], F32, kind="ExternalInput").ap()
    # hsall: 7 k-chunks of [hs(256); hs*rx(256); hs*ry(256); 1; rx; ry],
    # packed column-major in chunk order [0, 1, 6, 2, 3, 4, 5] so the
    # L2/L3-critical chunks (0, 1, 6) transfer first.
    hsall = nc.dram_tensor("hsall", [128, 700], F16, kind="ExternalInput").ap()
    # wall: [3 x 81 w23+w1 cols (slots 0-2)] then [7 x 88 w0 cols]
    wall = nc.dram_tensor("wall", [128, 859], F16, kind="ExternalInput").ap()
    cst = nc.dram_tensor("cst", [3, P], F32, kind="ExternalInput").ap()
    outp = nc.dram_tensor("outp", [NQ, P], F32, kind="ExternalOutput").ap()

    def relu_act(out_ap, in_ap):
        nc.scalar.activation(out_ap, in_ap, Relu)

    def relu_dve(out_ap, in_ap):
        nc.vector.tensor_scalar(out=out_ap, in0=in_ap, scalar1=0.0,
                                scalar2=None, op0=mybir.AluOpType.max)

    with tile.TileContext(nc) as tc:
        with tc.tile_pool(name="persist", bufs=1) as pers:
            # ---------------- persistent SBUF ----------------
            Ft = pers.tile([128, P], F32R, tag="F")
            HSB = pers.tile([128, 700], F16, tag="HSB")
            WSB = pers.tile([128, 859], F16, tag="WSB")
            PW0S = pers.tile([11, 800], F32, tag="PW0S")
            PW1S = pers.tile([9, 840], F32, tag="PW1S")
            PW23S = pers.tile([9, 105], F32, tag="PW23S")

            T1 = pers.tile([86, 840], F32R, tag="T1")
            T2 = pers.tile([121, 840], F32R, tag="T2")
            T3 = pers.tile([121, 705], F32R, tag="T3")
            Y1 = pers.tile([121, 7168], F32R, tag="Y1")   # (g,buf) col blocks
            Y2 = pers.tile([121, 17920], F32R, tag="Y2")
            STG = [pers.tile([NQ, 2048], F32, name=f"STG{i}", tag=f"STG{i}")
                   for i in range(2)]

            pstr = lambda t: t.ap[0][0]   # partition stride (elements)

            def y1slice(g, buf, cl, rows=121):
                base = (2 * g + buf) * CHUNK
                return Y1[0:rows, base:base + cl]

            def y2slice(g, buf, cl, rows=121):
                base = (5 * g + buf) * CHUNK
                return Y2[0:rows, base:base + cl]

            # ---------------- input DMAs ----------------
            nc.sync.dma_start(out=HSB[:, 0:300], in_=hsall[:, 0:300])
            nc.scalar.dma_start(out=WSB[:, 0:243], in_=wall[:, 0:243])
            nc.sync.dma_start(out=HSB[:, 300:700], in_=hsall[:, 300:700])
            nc.scalar.dma_start(out=WSB[:, 243:859], in_=wall[:, 243:859])

            # F layout: [0:8]=feat0, [8:10]=-gx,-gy, [10]=ones,
            #           [32:40]=feat1, [40:42]=-gx,-gy, [42]=ones,
            #           [64:72]=feat0, [72:74]=-gx,-gy, [74]=ones,
            #           [75:83]=feat1, [83:85]=-gx,-gy, [85]=ones
            nc.sync.dma_start(out=R(Ft[0:8, :]), in_=R(mf[0]))
            nc.sync.dma_start(out=R(Ft[8:11, :]), in_=R(cst[0:3, :]))
            nc.scalar.dma_start(out=R(Ft[32:40, :]), in_=R(mf[1]))
            nc.scalar.dma_start(out=R(Ft[40:43, :]), in_=R(cst[0:3, :]))
            # mixed band: SBUF->SBUF copies of the two main bands
            with tc.high_priority():
                nc.gpsimd.dma_start(out=R(Ft[64:75, :]), in_=R(Ft[0:11, :]))
                nc.gpsimd.dma_start(out=R(Ft[75:86, :]), in_=R(Ft[32:43, :]))
                # ones rows of Y1/Y2 (row 120) from the Ft ones row
                nc.gpsimd.dma_start(out=FV(Y1[120:121, :]),
                                    in_=FV(Ft[10:11, 0:7168]))
                nc.gpsimd.dma_start(out=FV(Y2[120:121, 0:8640]),
                                    in_=FV(Ft[10:11, 0:8640]))
                nc.gpsimd.dma_start(out=FV(Y2[120:121, 8640:17280]),
                                    in_=FV(Ft[10:11, 0:8640]))
                nc.gpsimd.dma_start(out=FV(Y2[120:121, 17280:17920]),
                                    in_=FV(Ft[10:11, 0:640]))

            # ---------------- zero fills ----------------
            nc.gpsimd.memset(FV(T2[:, :]), 0.0)
            nc.gpsimd.memset(FV(T3[:, :]), 0.0)
            nc.gpsimd.memset(FV(T1[64:86, 720:840]), 0.0)
            nc.gpsimd.memset(PW1S[0:9, 800:840], 0.0)
            nc.gpsimd.memset(PW23S[0:9, 100:105], 0.0)
            # mixed-group stale rows (80-119) of Y1/Y2 col blocks
            nc.gpsimd.memset(FV(Y1[64:120, 6144:7168]), 0.0)
            nc.gpsimd.memset(FV(Y2[64:120, 15360:17920]), 0.0)

            # ---------------- controller matmuls (fp16) ----------------
            with tc.tile_pool(name="psctrl", bufs=1, space="PSUM") as psc:
                pw1p = psc.tile([9, 1024], F32, tag="pw1p")
                pw23p = psc.tile([9, NQ], F32, tag="pw23p")
                pw0p = psc.tile([11, 1024], F32, tag="pw0p")
                # packed slot s holds logical chunk [0,1,6,2,3,4,5][s]
                # L2/L3 params contract over hs + tail chunks (slots 0,1,2)
                k3 = [(0, 128), (1, 128), (2, 3)]
                # L1 params (incl c row) contract over all 7 slots
                k7 = [(0, 128), (1, 128), (3, 128), (4, 128), (5, 128),
                      (6, 128), (2, 3)]
                # w1/b1 params first: T2 build is the long pole
                for i, (kc, kn) in enumerate(k3):
                    c0 = kc * 81
                    nc.tensor.matmul(
                        pw23p[0:9, 0:NQ],
                        WSB[0:kn, c0:c0 + 9],
                        HSB[0:kn, kc * 100:kc * 100 + NQ],
                        start=(i == 0), stop=(i == 2))
                for o2 in range(8):
                    for i, (kc, kn) in enumerate(k3):
                        c0 = kc * 81 + 9 + o2 * 9
                        nc.tensor.matmul(
                            pw1p[0:9, o2 * 128:o2 * 128 + NQ],
                            WSB[0:kn, c0:c0 + 9],
                            HSB[0:kn, kc * 100:kc * 100 + NQ],
                            start=(i == 0), stop=(i == 2))
                for o in range(8):
                    for i, (kc, kn) in enumerate(k7):
                        c0 = 243 + kc * 88 + o * 11
                        nc.tensor.matmul(
                            pw0p[0:11, o * 128:o * 128 + NQ],
                            WSB[0:kn, c0:c0 + 11],
                            HSB[0:kn, kc * 100:kc * 100 + NQ],
                            start=(i == 0), stop=(i == 6))

                # evacuate psum -> SBUF (the only way out of PSUM)
                nc.vector.tensor_copy(
                    PW1S[:, 0:800].rearrange("p (n o) -> p o n", o=8),
                    pw1p.rearrange("p (o n) -> p o n", o=8)[:, :, 0:NQ])
                nc.scalar.activation(PW23S[:, 0:NQ], pw23p[:, :], Copy)
                nc.scalar.activation(
                    PW0S.rearrange("p (o n) -> p o n", o=8),
                    pw0p.rearrange("p (o n) -> p o n", o=8)[:, :, 0:NQ], Copy)

            # ---------------- weight scatters ----------------
            # T2 per-j: dst [j*8:(j+1)*8, g*120+j*8+o2], src PW1S[o, (15g+j)*8+o2]
            dma_rot = [nc.sync, nc.sync, nc.sync, nc.gpsimd]
            for j in range(15):
                gcnt = 7 if j < 10 else 6
                if j % 4 == 0:
                    # 32-aligned partition base: engine copy
                    dst = T2[j * 8:j * 8 + 8, :].rearrange(
                        "p (g x) -> p g x", g=7)[:, 0:gcnt, j * 8:j * 8 + 8]
                    sv = PW1S[0:8, 0:840].rearrange(
                        "p (g x) -> p g x", g=7)[:, 0:gcnt, j * 8:j * 8 + 8]
                    if j % 8 == 0:
                        nc.vector.tensor_copy(dst, sv)
                    else:
                        nc.scalar.activation(dst, sv, Copy)
                else:
                    s = bass.AP(tensor=PW1S.tensor, offset=PW1S.offset + j * 8,
                                ap=[[pstr(PW1S), 8], [120, gcnt], [1, 8]])
                    dst = bass.AP(tensor=T2.tensor,
                                  offset=T2.offset + j * 8 * pstr(T2) + j * 8,
                                  ap=[[pstr(T2), 8], [120, gcnt], [1, 8]])
                    dma_rot[j % 4].dma_start(out=dst.bitcast(F32R),
                                             in_=s.bitcast(F32R))
            # T2 b1 row: dst [120, g*120+j*8+o2] <- PW1S[8, (15g+j)*8+o2]
            srcb1 = bass.AP(tensor=PW1S.tensor, offset=PW1S.offset + 8 * pstr(PW1S),
                            ap=[[pstr(PW1S), 1], [120, 7], [1, 120]])
            dstb1 = bass.AP(tensor=T2.tensor, offset=T2.offset + 120 * pstr(T2),
                            ap=[[pstr(T2), 1], [120, 7], [1, 120]])
            nc.sync.dma_start(out=dstb1.bitcast(F32R), in_=srcb1.bitcast(F32R))

            # T1 scatters (engine copies where partition base allows)
            # f0 block: rows 0-9 <- PW0S rows 0-9, cols (g,j,o) g-major
            src_f0 = bass.AP(tensor=PW0S.tensor, offset=PW0S.offset,
                             ap=[[pstr(PW0S), 11], [1, 45], [100, 8]])
            nc.vector.tensor_copy(T1[0:11, 0:360].rearrange(
                "p (n o) -> p n o", o=8), src_f0)
            src_f1 = bass.AP(tensor=PW0S.tensor, offset=PW0S.offset + 45,
                             ap=[[pstr(PW0S), 11], [1, 45], [100, 8]])
            nc.scalar.activation(T1[32:43, 360:720].rearrange(
                "p (n o) -> p n o", o=8), src_f1, Copy)
            # mixed f0-half rows 64-74 incl c (n' 90-94), base 64 ok
            src_m0 = bass.AP(tensor=PW0S.tensor, offset=PW0S.offset + 90,
                             ap=[[pstr(PW0S), 11], [1, 5], [100, 8]])
            nc.vector.tensor_copy(T1[64:75, 720:760].rearrange(
                "p (n o) -> p n o", o=8), src_m0)
            # mixed f1-half rows 75-85 incl c (n' 95-99): base 75 -> DMA/query
            for n in range(5):
                src_m1 = bass.AP(tensor=PW0S.tensor, offset=PW0S.offset + 95 + n,
                                 ap=[[pstr(PW0S), 11], [100, 8]])
                dst_m1 = bass.AP(tensor=T1.tensor,
                                 offset=T1.offset + 75 * pstr(T1) + 760 + n * 8,
                                 ap=[[pstr(T1), 11], [1, 8]])
                nc.sync.dma_start(out=dst_m1.bitcast(F32R),
                                  in_=src_m1.bitcast(F32R))

            # T3 per-j: dst [j*8+o rows, col 115g+j], src PW23S[o, 15g+j]
            dma_rot3 = [nc.sync, nc.gpsimd]
            for j in range(15):
                gcnt = 7 if j < 10 else 6
                s = bass.AP(tensor=PW23S.tensor, offset=PW23S.offset + j,
                            ap=[[pstr(PW23S), 8], [15, gcnt], [1, 1]])
                dst = bass.AP(tensor=T3.tensor,
                              offset=T3.offset + j * 8 * pstr(T3) + j,
                              ap=[[pstr(T3), 8], [115, gcnt], [1, 1]])
                dma_rot3[j % 2].dma_start(out=dst.bitcast(F32R),
                                          in_=s.bitcast(F32R))
            # T3 b2 row: dst [120, 115g+j], padded to 7 groups x 15
            srcb2a = bass.AP(tensor=PW23S.tensor,
                             offset=PW23S.offset + 8 * pstr(PW23S),
                             ap=[[pstr(PW23S), 1], [15, 7], [1, 15]])
            dstb2a = bass.AP(tensor=T3.tensor, offset=T3.offset + 120 * pstr(T3),
                             ap=[[pstr(T3), 1], [115, 7], [1, 15]])
            nc.gpsimd.dma_start(out=dstb2a.bitcast(F32R), in_=srcb2a.bitcast(F32R))

            # ---------------- main loop (skewed) ----------------
            psm_cm = tc.tile_pool(name="psmain", bufs=1, space="PSUM")
            psm = psm_cm.__enter__()
            ps1 = [psm.tile([120, CHUNK], F32, name=f"ps1_{i}", tag=f"ps1_{i}")
                   for i in range(4)]
            ps2 = [psm.tile([120, CHUNK], F32, name=f"ps2_{i}", tag=f"ps2_{i}")
                   for i in range(2)]
            ps3 = [psm.tile([NQ, CHUNK], F32, name=f"ps3_{i}", tag=f"ps3_{i}")
                   for i in range(2)]

            out_q = [nc.sync, nc.gpsimd]

            def emit_L3(cp):
                coff, clen = CHUNKS[cp]
                ph = ps3[cp % 2]
                for g in range(7):
                    nc.tensor.matmul(
                        ph[0:NQ, 0:clen],
                        T3[0:121, 100 * g:100 * g + NQ],
                        y2slice(g, cp % 5, clen),
                        start=(g == 0), stop=(g == 6),
                        skip_group_check=True)

            def emit_L3copy(cp):
                coff, clen = CHUNKS[cp]
                ph = ps3[cp % 2]
                qi = next(i for i, (a, b) in enumerate(QUARTERS) if a <= cp < b)
                q0 = CHUNKS[QUARTERS[qi][0]][0]
                stg = STG[qi % 2]
                nc.scalar.activation(stg[0:NQ, coff - q0:coff - q0 + clen],
                                     ph[0:NQ, 0:clen], Copy)
                if cp == QUARTERS[qi][1] - 1:
                    qlen = coff + clen - q0
                    last = (qi == len(QUARTERS) - 1)
                    qa = out_q[qi % 2]
                    qb = out_q[(qi + 1) % 2]
                    qc = nc.scalar if last else out_q[qi % 2]
                    qa.dma_start(out=outp[0:45, q0:q0 + qlen],
                                 in_=stg[0:45, 0:qlen])
                    qb.dma_start(out=outp[50:95, q0:q0 + qlen],
                                 in_=stg[45:90, 0:qlen])
                    qc.dma_start(out=outp[45:50, q0:q0 + qlen],
                                 in_=stg[90:95, 0:qlen])
                    qc.dma_start(out=outp[95:100, q0:q0 + qlen],
                                 in_=stg[95:100, 0:qlen])

            def emit_L1(ci, g):
                coff, clen = CHUNKS[ci]
                band, k1, nq = GROUPS[g]
                m = nq * 8
                pa = ps1[(ci * 7 + g) % 4]
                nc.tensor.matmul(
                    pa[0:m, 0:clen],
                    T1[band:band + k1, g * 120:g * 120 + m],
                    Ft[band:band + k1, coff:coff + clen],
                    start=True, stop=True)
                relu = relu_act if g % 2 == 0 else relu_dve
                relu(y1slice(g, ci % 2, clen, rows=m), pa[0:m, 0:clen])

            def emit_L2(ci, g):
                coff, clen = CHUNKS[ci]
                _, _, nq = GROUPS[g]
                m = nq * 8
                pb = ps2[(ci * 7 + g) % 2]
                nc.tensor.matmul(
                    pb[0:m, 0:clen],
                    T2[0:121, g * 120:g * 120 + m],
                    y1slice(g, ci % 2, clen),
                    start=True, stop=True)
                relu = relu_dve if g % 2 == 0 else relu_act
                relu(y2slice(g, ci % 5, clen, rows=m), pb[0:m, 0:clen])

            l3_next = [0]
            for ci in range(NCH + 2):
                pend = []
                if ci < NCH:
                    for g in range(3):
                        emit_L1(ci, g)
                if ci >= 5:
                    # catch-up: 2 chains while behind, and again at the tail
                    n_emit = 2 if (l3_next[0] < ci - 3 or ci >= NCH - 1) else 1
                    limit = ci - 1 if ci >= NCH - 1 else ci - 2
                    for _ in range(n_emit):
                        if l3_next[0] <= min(limit, NCH - 1):
                            emit_L3(l3_next[0])
                            pend.append(l3_next[0])
                            l3_next[0] += 1
                if ci < NCH:
                    for g in range(7):
                        emit_L2(ci, g)
                        if g + 3 < 7:
                            emit_L1(ci, g + 3)
                        if g == 0:
                            for cp in pend:
                                emit_L3copy(cp)
                            pend = []
                for cp in pend:
                    emit_L3copy(cp)
            while l3_next[0] < NCH:
                emit_L3(l3_next[0])
                emit_L3copy(l3_next[0])
                l3_next[0] += 1
            psm_cm.__exit__(None, None, None)

    nc.compile()
    return nc


_NC = None


def _get_nc():
    global _NC
    if _NC is None:
        _NC = _build_program()
    return _NC


# n' permutation: group-major query order per core
NPERM = np.concatenate([np.arange(0, 45), np.arange(50, 95),
                        np.arange(45, 50), np.arange(95, 100)])


def _host_pack(hs, mask_features, references, sizes, W_ctrl, b_ctrl):
    hs = np.asarray(hs, np.float32)
    mask_features = np.asarray(mask_features, np.float32)
    references = np.asarray(references, np.float32)
    sizes = np.asarray(sizes, np.float32)
    W_ctrl = np.asarray(W_ctrl, np.float32)
    b_ctrl = np.asarray(b_ctrl, np.float32)

    # pixel grid
    xs = np.arange(W, dtype=np.float32) * STRIDE + STRIDE // 2
    ys = np.arange(H, dtype=np.float32) * STRIDE + STRIDE // 2
    gxf = np.tile(xs, H)
    gyf = np.repeat(ys, W)
    cstm = np.stack([-gxf, -gyf, np.ones(P, np.float32)]).astype(np.float32)

    # wall [771, 169]: rows contract against [hs; hs*rx; hs*ry; 1; rx; ry]
    W_aug = np.concatenate([W_ctrl.T, b_ctrl[None, :]], 0)  # [257, 169]
    wall = np.zeros((771, 169), np.float32)

    def setcol(c, p):
        wall[0:256, c] = W_aug[0:256, p]
        wall[768, c] = W_aug[256, p]

    for o in range(8):
        for i in range(10):
            setcol(o * 11 + i, o * 10 + i)
        c = o * 11 + 10                      # c = w8*rx + w9*ry + b0
        wall[256:512, c] = W_aug[0:256, o * 10 + 8]
        wall[769, c] = W_aug[256, o * 10 + 8]
        wall[512:768, c] = W_aug[0:256, o * 10 + 9]
        wall[770, c] = W_aug[256, o * 10 + 9]
        wall[0:256, c] = W_aug[0:256, 152 + o]
        wall[768, c] = W_aug[256, 152 + o]
    for o2 in range(8):
        for j in range(8):
            setcol(88 + o2 * 9 + j, 80 + o2 * 8 + j)
        setcol(88 + o2 * 9 + 8, 160 + o2)
    for j in range(8):
        setcol(160 + j, 144 + j)
    setcol(168, 168)
    # packed slot s holds logical k-chunk [0, 1, 6(tail), 2, 3, 4, 5][s]:
    # first 3x81 = w2/b2 + w1/b1 cols of slots 0-2, then 7x88 w0 cols
    wpk = np.zeros((128, 859), np.float32)
    for s, kc in enumerate([0, 1, None, 2, 3, 4, 5]):
        rows = wall[768:771] if kc is None else wall[128 * kc:128 * (kc + 1)]
        nr = rows.shape[0]
        if s < 3:
            wpk[0:nr, s * 81:s * 81 + 9] = rows[:, 160:169]
            wpk[0:nr, s * 81 + 9:s * 81 + 81] = rows[:, 88:160]
        wpk[0:nr, 243 + s * 88:243 + s * 88 + 88] = rows[:, 0:88]
    wall = wpk.astype(np.float16)

    # reference points in pixels
    b_idx = np.arange(16) // 8
    scale = sizes[b_idx][:, ::-1]                      # [16, 2] = (img_w, img_h)
    refs_px = references * scale[:, None, :]           # [16, 50, 2]

    in_maps = []
    for c in range(NCORES):
        hs_c = hs[2 * c:2 * c + 2].reshape(NQ, HID)[NPERM]
        mf_c = mask_features[2 * c:2 * c + 2].reshape(2, 8, P)
        rp = refs_px[2 * c:2 * c + 2].reshape(NQ, 2)[NPERM]
        hrows = np.empty((771, NQ), np.float32)
        hrows[0:256] = hs_c.T
        hrows[256:512] = hs_c.T * rp[:, 0][None, :]
        hrows[512:768] = hs_c.T * rp[:, 1][None, :]
        hrows[768] = 1.0
        hrows[769] = rp[:, 0]
        hrows[770] = rp[:, 1]
        hpk = np.zeros((128, 700), np.float32)
        for s, kc in enumerate([0, 1, None, 2, 3, 4, 5]):
            if kc is None:
                hpk[0:3, s * 100:s * 100 + 100] = hrows[768:771]
            else:
                hpk[:, s * 100:s * 100 + 100] = hrows[128 * kc:128 * (kc + 1)]
        in_maps.append(dict(
            mf=np.ascontiguousarray(mf_c),
            hsall=hpk.astype(np.float16),
            wall=wall,
            cst=cstm,
        ))
    return in_maps


def kernel(hs, mask_features, references, sizes, W_ctrl, b_ctrl, T):
    assert int(T) == 8
    nc = _get_nc()
    in_maps = _host_pack(hs, mask_features, references, sizes, W_ctrl, b_ctrl)
    res = bass_utils.run_bass_kernel_spmd(nc, in_maps, core_ids=list(range(NCORES)))
    out = np.empty((16, Q, H, W), np.float32)
    for c in range(NCORES):
        out[2 * c:2 * c + 2] = res.results[c]["outp"].reshape(2, Q, H, W)
    return out


# revision 10
# speedup vs baseline: 1.0219x; 1.0014x over previous
"""DynamicSegmentationHead Trainium2 kernel, restructured for overlap.

Data-parallel over 16 clip-frames: each core handles 2 frames (100 queries).
Query order is host-permuted to group-major n' = [f0 q0-44, f1 q0-44,
f0 q45-49, f1 q45-49] so every weight-scatter is one uniform strided DMA
and the L3 psum block maps to output rows with 3 clean DMAs per flush.

Controller runs in fp16 over an extended K of 771: host supplies
[hs; hs*refx; hs*refy; ones; refx; refy] so the per-query constant row
c = w8*refx + w9*refy + b0 falls out of the matmul directly (no separate
c-row pipeline). pw23 -> pw1 -> pw0 ordering releases the T3/T2/T1 build
paths in deadline order.

Per core, per 512-pixel chunk (17 chunks):
  L1: y1 = relu(T1 @ Ft)    7 matmuls K=11/22, M=120/80, psum rot x3
  L2: y2 = relu(T2 @ y1)    7 matmuls K=121 blockdiag, psum rot x3
  L3: one [100,512] psum, 7 accumulating matmuls, emitted with a
      progressive 2-chunk skew (first chains deferred to iteration 4 so
      they never wait on the T3 scatter DMAs), Y2 5-deep
  ACT stage copy + output flush DMAs per span, relus split ACT/DVE.

Steady state is ACT/DVE-bound: 15 psum-evacuation ops per chunk across
2 engines is the structural floor (PSUM has 8 banks; DMA cannot read
PSUM; 7 query groups is minimal since 800 M-rows / 128 partitions).
"""

import numpy as np

import concourse.bass as bass
import concourse.bacc as bacc
import concourse.tile as tile
from concourse import mybir
from concourse import bass_utils

F32 = mybir.dt.float32
F32R = mybir.dt.float32r
F16 = mybir.dt.float16

HID = 256
Q = 50
H, W = 72, 120
P = H * W            # 8640
NQ = 100             # queries per core (2 frames)
NCORES = 8
STRIDE = 4

CHUNK = 512
NCH = 17
CHUNKS = [(i * CHUNK, CHUNK) for i in range(16)] + [(16 * CHUNK, P - 16 * CHUNK)]
QUARTERS = [(0, 4), (4, 8), (8, 12), (12, 15), (15, 17)]   # flush spans

# groups in n' (group-major) order: 3 f0, 3 f1, 1 mixed
# (band, k1, nq); n'-base of group g is 15*g
GROUPS = [(0, 11, 15)] * 3 + [(32, 11, 15)] * 3 + [(64, 22, 10)]

Relu = mybir.ActivationFunctionType.Relu
Copy = mybir.ActivationFunctionType.Copy


def _build_program():
    nc = bacc.Bacc("TRN2", target_bir_lowering=False, debug=False)
    R = lambda ap: ap.bitcast(F32R)
    FV = lambda ap: ap.bitcast(F32)

    mf = nc.dram_tensor("mf", [2, 8, P], F32, kind="ExternalInput").ap()
    # hsall: 7 k-chunks of [hs(256); hs*rx(256); hs*ry(256); 1; rx; ry],
    # packed column-major in chunk order [0, 1, 6, 2, 3, 4, 5] so the
    # L2/L3-critical chunks (0, 1, 6) transfer first.
    hsall = nc.dram_tensor("hsall", [128, 700], F16, kind="ExternalInput").ap()
    # wall: [3 x 81 w23+w1 cols (slots 0-2)] then [7 x 88 w0 cols]
    wall = nc.dram_tensor("wall", [128, 859], F16, kind="ExternalInput").ap()
    cst = nc.dram_tensor("cst", [3, P], F32, kind="ExternalInput").ap()
    outp = nc.dram_tensor("outp", [NQ, P], F32, kind="ExternalOutput").ap()

    def relu_act(out_ap, in_ap):
        nc.scalar.activation(out_ap, in_ap, Relu)

    def relu_dve(out_ap, in_ap):
        nc.vector.tensor_scalar(out=out_ap, in0=in_ap, scalar1=0.0,
                                scalar2=None, op0=mybir.AluOpType.max)

    with tile.TileContext(nc) as tc:
        with tc.tile_pool(name="persist", bufs=1) as pers:
            # ---------------- persistent SBUF ----------------
            Ft = pers.tile([128, P], F32R, tag="F")
            HSB = pers.tile([128, 700], F16, tag="HSB")
            WSB = pers.tile([128, 859], F16, tag="WSB")
            PW0S = pers.tile([11, 800], F32, tag="PW0S")
            PW1S = pers.tile([9, 840], F32, tag="PW1S")
            PW23S = pers.tile([9, 105], F32, tag="PW23S")

            T1 = pers.tile([86, 840], F32R, tag="T1")
            T2 = pers.tile([121, 840], F32R, tag="T2")
            T3 = pers.tile([121, 705], F32R, tag="T3")
            Y1 = pers.tile([121, 7168], F32R, tag="Y1")   # (g,buf) col blocks
            Y2 = pers.tile([121, 17920], F32R, tag="Y2")
            STG = [pers.tile([NQ, 2048], F32, name=f"STG{i}", tag=f"STG{i}")
                   for i in range(2)]

            pstr = lambda t: t.ap[0][0]   # partition stride (elements)

            def y1slice(g, buf, cl, rows=121):
                base = (2 * g + buf) * CHUNK
                return Y1[0:rows, base:base + cl]

            def y2slice(g, buf, cl, rows=121):
                base = (5 * g + buf) * CHUNK
                return Y2[0:rows, base:base + cl]

            # ---------------- input DMAs ----------------
            nc.sync.dma_start(out=HSB[:, 0:300], in_=hsall[:, 0:300])
            nc.scalar.dma_start(out=WSB[:, 0:243], in_=wall[:, 0:243])
            nc.sync.dma_start(out=HSB[:, 300:700], in_=hsall[:, 300:700])
            nc.scalar.dma_start(out=WSB[:, 243:859], in_=wall[:, 243:859])

            # F layout: [0:8]=feat0, [8:10]=-gx,-gy, [10]=ones,
            #           [32:40]=feat1, [40:42]=-gx,-gy, [42]=ones,
            #           [64:72]=feat0, [72:74]=-gx,-gy, [74]=ones,
            #           [75:83]=feat1, [83:85]=-gx,-gy, [85]=ones
            nc.sync.dma_start(out=R(Ft[0:8, :]), in_=R(mf[0]))
            nc.sync.dma_start(out=R(Ft[8:11, :]), in_=R(cst[0:3, :]))
            nc.scalar.dma_start(out=R(Ft[32:40, :]), in_=R(mf[1]))
            nc.scalar.dma_start(out=R(Ft[40:43, :]), in_=R(cst[0:3, :]))
            # mixed band: SBUF->SBUF copies of the two main bands
            with tc.high_priority():
                nc.gpsimd.dma_start(out=R(Ft[64:75, :]), in_=R(Ft[0:11, :]))
                nc.gpsimd.dma_start(out=R(Ft[75:86, :]), in_=R(Ft[32:43, :]))
                # ones rows of Y1/Y2 (row 120) from the Ft ones row
                nc.gpsimd.dma_start(out=FV(Y1[120:121, :]),
                                    in_=FV(Ft[10:11, 0:7168]))
                nc.gpsimd.dma_start(out=FV(Y2[120:121, 0:8640]),
                                    in_=FV(Ft[10:11, 0:8640]))
                nc.gpsimd.dma_start(out=FV(Y2[120:121, 8640:17280]),
                                    in_=FV(Ft[10:11, 0:8640]))
                nc.gpsimd.dma_start(out=FV(Y2[120:121, 17280:17920]),
                                    in_=FV(Ft[10:11, 0:640]))

            # ---------------- zero fills ----------------
            nc.gpsimd.memset(FV(T2[:, :]), 0.0)
            nc.gpsimd.memset(FV(T3[:, :]), 0.0)
            nc.gpsimd.memset(FV(T1[64:86, 720:840]), 0.0)
            nc.gpsimd.memset(PW1S[0:9, 800:840], 0.0)
            nc.gpsimd.memset(PW23S[0:9, 100:105], 0.0)
            # mixed-group stale rows (80-119) of Y1/Y2 col blocks
            nc.gpsimd.memset(FV(Y1[64:120, 6144:7168]), 0.0)
            nc.gpsimd.memset(FV(Y2[64:120, 15360:17920]), 0.0)

            # ---------------- controller matmuls (fp16) ----------------
            with tc.tile_pool(name="psctrl", bufs=1, space="PSUM") as psc:
                pw1p = psc.tile([9, 1024], F32, tag="pw1p")
                pw23p = psc.tile([9, NQ], F32, tag="pw23p")
                pw0p = psc.tile([11, 1024], F32, tag="pw0p")
                # packed slot s holds logical chunk [0,1,6,2,3,4,5][s]
                # L2/L3 params contract over hs + tail chunks (slots 0,1,2)
                k3 = [(0, 128), (1, 128), (2, 3)]
                # L1 params (incl c row) contract over all 7 slots
                k7 = [(0, 128), (1, 128), (3, 128), (4, 128), (5, 128),
                      (6, 128), (2, 3)]
                # w1/b1 params first: T2 build is the long pole
                for i, (kc, kn) in enumerate(k3):
                    c0 = kc * 81
                    nc.tensor.matmul(
                        pw23p[0:9, 0:NQ],
                        WSB[0:kn, c0:c0 + 9],
                        HSB[0:kn, kc * 100:kc * 100 + NQ],
                        start=(i == 0), stop=(i == 2))
                for o2 in range(8):
                    for i, (kc, kn) in enumerate(k3):
                        c0 = kc * 81 + 9 + o2 * 9
                        nc.tensor.matmul(
                            pw1p[0:9, o2 * 128:o2 * 128 + NQ],
                            WSB[0:kn, c0:c0 + 9],
                            HSB[0:kn, kc * 100:kc * 100 + NQ],
                            start=(i == 0), stop=(i == 2))
                for o in range(8):
                    for i, (kc, kn) in enumerate(k7):
                        c0 = 243 + kc * 88 + o * 11
                        nc.tensor.matmul(
                            pw0p[0:11, o * 128:o * 128 + NQ],
                            WSB[0:kn, c0:c0 + 11],
                            HSB[0:kn, kc * 100:kc * 100 + NQ],
                            start=(i == 0), stop=(i == 6))

                # evacuate psum -> SBUF (the only way out of PSUM)
                nc.vector.tensor_copy(
                    PW1S[:, 0:800].rearrange("p (n o) -> p o n", o=8),
                    pw1p.rearrange("p (o n) -> p o n", o=8)[:, :, 0:NQ])
                nc.scalar.activation(PW23S[:, 0:NQ], pw23p[:, :], Copy)
                nc.scalar.activation(
                    PW0S.rearrange("p (o n) -> p o n", o=8),
                    pw0p.rearrange("p (o n) -> p o n", o=8)[:, :, 0:NQ], Copy)

            # ---------------- weight scatters ----------------
            # T2 per-j: dst [j*8:(j+1)*8, g*120+j*8+o2], src PW1S[o, (15g+j)*8+o2]
            dma_rot = [nc.sync, nc.sync, nc.sync, nc.gpsimd]
            for j in range(15):
                gcnt = 7 if j < 10 else 6
                if j % 4 == 0:
                    # 32-aligned partition base: engine copy
                    dst = T2[j * 8:j * 8 + 8, :].rearrange(
                        "p (g x) -> p g x", g=7)[:, 0:gcnt, j * 8:j * 8 + 8]
                    sv = PW1S[0:8, 0:840].rearrange(
                        "p (g x) -> p g x", g=7)[:, 0:gcnt, j * 8:j * 8 + 8]
                    if j % 8 == 0:
                        nc.vector.tensor_copy(dst, sv)
                    else:
                        nc.scalar.activation(dst, sv, Copy)
                else:
                    s = bass.AP(tensor=PW1S.tensor, offset=PW1S.offset + j * 8,
                                ap=[[pstr(PW1S), 8], [120, gcnt], [1, 8]])
                    dst = bass.AP(tensor=T2.tensor,
                                  offset=T2.offset + j * 8 * pstr(T2) + j * 8,
                                  ap=[[pstr(T2), 8], [120, gcnt], [1, 8]])
                    dma_rot[j % 4].dma_start(out=dst.bitcast(F32R),
                                             in_=s.bitcast(F32R))
            # T2 b1 row: dst [120, g*120+j*8+o2] <- PW1S[8, (15g+j)*8+o2]
            srcb1 = bass.AP(tensor=PW1S.tensor, offset=PW1S.offset + 8 * pstr(PW1S),
                            ap=[[pstr(PW1S), 1], [120, 7], [1, 120]])
            dstb1 = bass.AP(tensor=T2.tensor, offset=T2.offset + 120 * pstr(T2),
                            ap=[[pstr(T2), 1], [120, 7], [1, 120]])
            nc.sync.dma_start(out=dstb1.bitcast(F32R), in_=srcb1.bitcast(F32R))

            # T1 scatters (engine copies where partition base allows)
            # f0 block: rows 0-9 <- PW0S rows 0-9, cols (g,j,o) g-major
            src_f0 = bass.AP(tensor=PW0S.tensor, offset=PW0S.offset,
                             ap=[[pstr(PW0S), 11], [1, 45], [100, 8]])
            nc.vector.tensor_copy(T1[0:11, 0:360].rearrange(
                "p (n o) -> p n o", o=8), src_f0)
            src_f1 = bass.AP(tensor=PW0S.tensor, offset=PW0S.offset + 45,
                             ap=[[pstr(PW0S), 11], [1, 45], [100, 8]])
            nc.scalar.activation(T1[32:43, 360:720].rearrange(
                "p (n o) -> p n o", o=8), src_f1, Copy)
            # mixed f0-half rows 64-74 incl c (n' 90-94), base 64 ok
            src_m0 = bass.AP(tensor=PW0S.tensor, offset=PW0S.offset + 90,
                             ap=[[pstr(PW0S), 11], [1, 5], [100, 8]])
            nc.vector.tensor_copy(T1[64:75, 720:760].rearrange(
                "p (n o) -> p n o", o=8), src_m0)
            # mixed f1-half rows 75-85 incl c (n' 95-99): base 75 -> DMA/query
            for n in range(5):
                src_m1 = bass.AP(tensor=PW0S.tensor, offset=PW0S.offset + 95 + n,
                                 ap=[[pstr(PW0S), 11], [100, 8]])
                dst_m1 = bass.AP(tensor=T1.tensor,
                                 offset=T1.offset + 75 * pstr(T1) + 760 + n * 8,
                                 ap=[[pstr(T1), 11], [1, 8]])
                nc.sync.dma_start(out=dst_m1.bitcast(F32R),
                                  in_=src_m1.bitcast(F32R))

            # T3 per-j: dst [j*8+o rows, col 115g+j], src PW23S[o, 15g+j]
            dma_rot3 = [nc.sync, nc.gpsimd]
            for j in range(15):
                gcnt = 7 if j < 10 else 6
                s = bass.AP(tensor=PW23S.tensor, offset=PW23S.offset + j,
                            ap=[[pstr(PW23S), 8], [15, gcnt], [1, 1]])
                dst = bass.AP(tensor=T3.tensor,
                              offset=T3.offset + j * 8 * pstr(T3) + j,
                              ap=[[pstr(T3), 8], [115, gcnt], [1, 1]])
                dma_rot3[j % 2].dma_start(out=dst.bitcast(F32R),
                                          in_=s.bitcast(F32R))
            # T3 b2 row: dst [120, 115g+j], padded to 7 groups x 15
            srcb2a = bass.AP(tensor=PW23S.tensor,
                             offset=PW23S.offset + 8 * pstr(PW23S),
                             ap=[[pstr(PW23S), 1], [15, 7], [1, 15]])
            dstb2a = bass.AP(tensor=T3.tensor, offset=T3.offset + 120 * pstr(T3),
                             ap=[[pstr(T3), 1], [115, 7], [1, 15]])
            nc.gpsimd.dma_start(out=dstb2a.bitcast(F32R), in_=srcb2a.bitcast(F32R))

            # ---------------- main loop (skewed) ----------------
            psm_cm = tc.tile_pool(name="psmain", bufs=1, space="PSUM")
            psm = psm_cm.__enter__()
            ps1 = [psm.tile([120, CHUNK], F32, name=f"ps1_{i}", tag=f"ps1_{i}")
                   for i in range(4)]
            ps2 = [psm.tile([120, CHUNK], F32, name=f"ps2_{i}", tag=f"ps2_{i}")
                   for i in range(2)]
            ps3 = [psm.tile([NQ, CHUNK], F32, name=f"ps3_{i}", tag=f"ps3_{i}")
                   for i in range(2)]

            out_q = [nc.sync, nc.gpsimd]

            def emit_L3(cp):
                coff, clen = CHUNKS[cp]
                ph = ps3[cp % 2]
                for g in range(7):
                    nc.tensor.matmul(
                        ph[0:NQ, 0:clen],
                        T3[0:121, 100 * g:100 * g + NQ],
                        y2slice(g, cp % 5, clen),
                        start=(g == 0), stop=(g == 6),
                        skip_group_check=True)

            def emit_L3copy(cp):
                coff, clen = CHUNKS[cp]
                ph = ps3[cp % 2]
                qi = next(i for i, (a, b) in enumerate(QUARTERS) if a <= cp < b)
                q0 = CHUNKS[QUARTERS[qi][0]][0]
                stg = STG[qi % 2]
                nc.scalar.activation(stg[0:NQ, coff - q0:coff - q0 + clen],
                                     ph[0:NQ, 0:clen], Copy)
                if cp == QUARTERS[qi][1] - 1:
                    qlen = coff + clen - q0
                    last = (qi == len(QUARTERS) - 1)
                    qa = out_q[qi % 2]
                    qb = out_q[(qi + 1) % 2]
                    qc = nc.scalar if last else out_q[qi % 2]
                    qa.dma_start(out=outp[0:45, q0:q0 + qlen],
                                 in_=stg[0:45, 0:qlen])
                    qb.dma_start(out=outp[50:95, q0:q0 + qlen],
                                 in_=stg[45:90, 0:qlen])
                    qc.dma_start(out=outp[45:50, q0:q0 + qlen],
                                 in_=stg[90:95, 0:qlen])
                    qc.dma_start(out=outp[95:100, q0:q0 + qlen],
                                 in_=stg[95:100, 0:qlen])

            def emit_L1(ci, g):
                coff, clen = CHUNKS[ci]
                band, k1, nq = GROUPS[g]
                m = nq * 8
                pa = ps1[(ci * 7 + g) % 4]
                nc.tensor.matmul(
                    pa[0:m, 0:clen],
                    T1[band:band + k1, g * 120:g * 120 + m],
                    Ft[band:band + k1, coff:coff + clen],
                    start=True, stop=True)
                relu = relu_act if g % 2 == 0 else relu_dve
                relu(y1slice(g, ci % 2, clen, rows=m), pa[0:m, 0:clen])

            def emit_L2(ci, g):
                coff, clen = CHUNKS[ci]
                _, _, nq = GROUPS[g]
                m = nq * 8
                pb = ps2[(ci * 7 + g) % 2]
                nc.tensor.matmul(
                    pb[0:m, 0:clen],
                    T2[0:121, g * 120:g * 120 + m],
                    y1slice(g, ci % 2, clen),
                    start=True, stop=True)
                relu = relu_dve if g % 2 == 0 else relu_act
                relu(y2slice(g, ci % 5, clen, rows=m), pb[0:m, 0:clen])

            l3_next = [0]
            for ci in range(NCH + 2):
                pend = []
                if ci < NCH:
                    for g in range(3):
                        emit_L1(ci, g)
                if ci >= 5:
                    # catch-up: 2 chains while behind, and again at the tail
                    n_emit = 2 if (l3_next[0] < ci - 3 or ci >= NCH - 1) else 1
                    limit = ci - 1 if ci >= NCH - 1 else ci - 2
                    for _ in range(n_emit):
                        if l3_next[0] <= min(limit, NCH - 1):
                            emit_L3(l3_next[0])
                            pend.append(l3_next[0])
                            l3_next[0] += 1
                if ci < NCH:
                    for g in range(7):
                        emit_L2(ci, g)
                        if g + 3 < 7:
                            emit_L1(ci, g + 3)
                        if g == 0:
                            for cp in pend:
                                emit_L3copy(cp)
                            pend = []
                for cp in pend:
                    emit_L3copy(cp)
            while l3_next[0] < NCH:
                emit_L3(l3_next[0])
                emit_L3copy(l3_next[0])
                l3_next[0] += 1
            psm_cm.__exit__(None, None, None)

    nc.compile()
    return nc


_NC = None


def _get_nc():
    global _NC
    if _NC is None:
        _NC = _build_program()
    return _NC


# n' permutation: group-major query order per core
NPERM = np.concatenate([np.arange(0, 45), np.arange(50, 95),
                        np.arange(45, 50), np.arange(95, 100)])


def _host_pack(hs, mask_features, references, sizes, W_ctrl, b_ctrl):
    hs = np.asarray(hs, np.float32)
    mask_features = np.asarray(mask_features, np.float32)
    references = np.asarray(references, np.float32)
    sizes = np.asarray(sizes, np.float32)
    W_ctrl = np.asarray(W_ctrl, np.float32)
    b_ctrl = np.asarray(b_ctrl, np.float32)

    # pixel grid
    xs = np.arange(W, dtype=np.float32) * STRIDE + STRIDE // 2
    ys = np.arange(H, dtype=np.float32) * STRIDE + STRIDE // 2
    gxf = np.tile(xs, H)
    gyf = np.repeat(ys, W)
    cstm = np.stack([-gxf, -gyf, np.ones(P, np.float32)]).astype(np.float32)

    # wall [771, 169]: rows contract against [hs; hs*rx; hs*ry; 1; rx; ry]
    W_aug = np.concatenate([W_ctrl.T, b_ctrl[None, :]], 0)  # [257, 169]
    wall = np.zeros((771, 169), np.float32)

    def setcol(c, p):
        wall[0:256, c] = W_aug[0:256, p]
        wall[768, c] = W_aug[256, p]

    for o in range(8):
        for i in range(10):
            setcol(o * 11 + i, o * 10 + i)
        c = o * 11 + 10                      # c = w8*rx + w9*ry + b0
        wall[256:512, c] = W_aug[0:256, o * 10 + 8]
        wall[769, c] = W_aug[256, o * 10 + 8]
        wall[512:768, c] = W_aug[0:256, o * 10 + 9]
        wall[770, c] = W_aug[256, o * 10 + 9]
        wall[0:256, c] = W_aug[0:256, 152 + o]
        wall[768, c] = W_aug[256, 152 + o]
    for o2 in range(8):
        for j in range(8):
            setcol(88 + o2 * 9 + j, 80 + o2 * 8 + j)
        setcol(88 + o2 * 9 + 8, 160 + o2)
    for j in range(8):
        setcol(160 + j, 144 + j)
    setcol(168, 168)
    # packed slot s holds logical k-chunk [0, 1, 6(tail), 2, 3, 4, 5][s]:
    # first 3x81 = w2/b2 + w1/b1 cols of slots 0-2, then 7x88 w0 cols
    wpk = np.zeros((128, 859), np.float32)
    for s, kc in enumerate([0, 1, None, 2, 3, 4, 5]):
        rows = wall[768:771] if kc is None else wall[128 * kc:128 * (kc + 1)]
        nr = rows.shape[0]
        if s < 3:
            wpk[0:nr, s * 81:s * 81 + 9] = rows[:, 160:169]
            wpk[0:nr, s * 81 + 9:s * 81 + 81] = rows[:, 88:160]
        wpk[0:nr, 243 + s * 88:243 + s * 88 + 88] = rows[:, 0:88]
    wall = wpk.astype(np.float16)

    # reference points in pixels
    b_idx = np.arange(16) // 8
    scale = sizes[b_idx][:, ::-1]                      # [16, 2] = (img_w, img_h)
    refs_px = references * scale[:, None, :]           # [16, 50, 2]

    in_maps = []
    for c in range(NCORES):
        hs_c = hs[2 * c:2 * c + 2].reshape(NQ, HID)[NPERM]
        mf_c = mask_features[2 * c:2 * c + 2].reshape(2, 8, P)
        rp = refs_px[2 * c:2 * c + 2].reshape(NQ, 2)[NPERM]
        hrows = np.empty((771, NQ), np.float32)
        hrows[0:256] = hs_c.T
        hrows[256:512] = hs_c.T * rp[:, 0][None, :]
        hrows[512:768] = hs_c.T * rp[:, 1][None, :]
        hrows[768] = 1.0
        hrows[769] = rp[:, 0]
        hrows[770] = rp[:, 1]
        hpk = np.zeros((128, 700), np.float32)
        for s, kc in enumerate([0, 1, None, 2, 3, 4, 5]):
            if kc is None:
                hpk[0:3, s * 100:s * 100 + 100] = hrows[768:771]
            else:
                hpk[:, s * 100:s * 100 + 100] = hrows[128 * kc:128 * (kc + 1)]
        in_maps.append(dict(
            mf=np.ascontiguousarray(mf_c),
            hsall=hpk.astype(np.float16),
            wall=wall,
            cst=cstm,
        ))
    return in_maps


def kernel(hs, mask_features, references, sizes, W_ctrl, b_ctrl, T):
    assert int(T) == 8
    nc = _get_nc()
    in_maps = _host_pack(hs, mask_features, references, sizes, W_ctrl, b_ctrl)
    res = bass_utils.run_bass_kernel_spmd(nc, in_maps, core_ids=list(range(NCORES)))
    out = np.empty((16, Q, H, W), np.float32)
    for c in range(NCORES):
        out[2 * c:2 * c + 2] = res.results[c]["outp"].reshape(2, Q, H, W)
    return out


# revision 12
# speedup vs baseline: 1.0508x; 1.0283x over previous
"""DynamicSegmentationHead Trainium2 kernel, restructured for overlap.

Data-parallel over 16 clip-frames: each core handles 2 frames (100 queries).
Query order is host-permuted to group-major n' = [f0 q0-44, f1 q0-44,
f0 q45-49, f1 q45-49] so every weight-scatter is one uniform strided DMA
and the L3 psum block maps to output rows with 3 clean DMAs per flush.

Controller runs in fp16 over an extended K of 771: host supplies
[hs; hs*refx; hs*refy; ones; refx; refy] so the per-query constant row
c = w8*refx + w9*refy + b0 falls out of the matmul directly (no separate
c-row pipeline). pw23 -> pw1 -> pw0 ordering releases the T3/T2/T1 build
paths in deadline order.

Per core, per 512-pixel chunk (17 chunks):
  L1: y1 = relu(T1 @ Ft)    7 matmuls K=11/22, M=120/80, psum rot x3
  L2: y2 = relu(T2 @ y1)    7 matmuls K=121 blockdiag, psum rot x3
  L3: one [100,512] psum x2, 7 accumulating matmuls, emitted with a
      progressive 2-chunk skew (first chains deferred to iteration 3 so
      they never wait on the T3 scatter DMAs), Y2 5-deep
  PSUM evacuations of the controller params split across ACT+DVE halves
  ACT stage copy + output flush DMAs per span, relus split ACT/DVE.

Steady state is ACT/DVE-bound: 15 psum-evacuation ops per chunk across
2 engines is the structural floor (PSUM has 8 banks; DMA cannot read
PSUM; 7 query groups is minimal since 800 M-rows / 128 partitions).
"""

import numpy as np

import concourse.bass as bass
import concourse.bacc as bacc
import concourse.tile as tile
from concourse import mybir
from concourse import bass_utils

F32 = mybir.dt.float32
F32R = mybir.dt.float32r
F16 = mybir.dt.float16

HID = 256
Q = 50
H, W = 72, 120
P = H * W            # 8640
NQ = 100             # queries per core (2 frames)
NCORES = 8
STRIDE = 4

CHUNK = 512
NCH = 17
CHUNKS = [(i * CHUNK, CHUNK) for i in range(16)] + [(16 * CHUNK, P - 16 * CHUNK)]
QUARTERS = [(0, 4), (4, 8), (8, 12), (12, 16), (16, 17)]   # flush spans

# groups in n' (group-major) order: 3 f0, 3 f1, 1 mixed
# (band, k1, nq); n'-base of group g is 15*g
GROUPS = [(0, 11, 15)] * 3 + [(32, 11, 15)] * 3 + [(64, 22, 10)]

Relu = mybir.ActivationFunctionType.Relu
Copy = mybir.ActivationFunctionType.Copy


def _build_program():
    nc = bacc.Bacc("TRN2", target_bir_lowering=False, debug=False)
    R = lambda ap: ap.bitcast(F32R)
    FV = lambda ap: ap.bitcast(F32)

    mf = nc.dram_tensor("mf", [2, 8, P], F32, kind="ExternalInput").ap()
    # hsall: 7 k-chunks of [hs(256); hs*rx(256); hs*ry(256); 1; rx; ry],
    # packed column-major in chunk order [0, 1, 6, 2, 3, 4, 5] so the
    # L2/L3-critical chunks (0, 1, 6) transfer first.
    hsall = nc.dram_tensor("hsall", [128, 700], F16, kind="ExternalInput").ap()
    # wall: [3 x 81 w23+w1 cols (slots 0-2)] then [7 x 88 w0 cols]
    wall = nc.dram_tensor("wall", [128, 859], F16, kind="ExternalInput").ap()
    cst = nc.dram_tensor("cst", [3, P], F32, kind="ExternalInput").ap()
    outp = nc.dram_tensor("outp", [NQ, P], F32, kind="ExternalOutput").ap()

    def relu_act(out_ap, in_ap):
        nc.scalar.activation(out_ap, in_ap, Relu)

    def relu_dve(out_ap, in_ap):
        nc.vector.tensor_scalar(out=out_ap, in0=in_ap, scalar1=0.0,
                                scalar2=None, op0=mybir.AluOpType.max)

    with tile.TileContext(nc) as tc:
        with tc.tile_pool(name="persist", bufs=1) as pers:
            # ---------------- persistent SBUF ----------------
            Ft = pers.tile([128, P], F32R, tag="F")
            HSB = pers.tile([128, 700], F16, tag="HSB")
            WSB = pers.tile([128, 859], F16, tag="WSB")
            PW0S = pers.tile([11, 800], F32, tag="PW0S")
            PW1S = pers.tile([9, 840], F32, tag="PW1S")
            PW23S = pers.tile([9, 105], F32, tag="PW23S")

            T1 = pers.tile([86, 840], F32R, tag="T1")
            T2 = pers.tile([121, 840], F32R, tag="T2")
            T3 = pers.tile([121, 705], F32R, tag="T3")
            Y1 = pers.tile([121, 7168], F32R, tag="Y1")   # (g,buf) col blocks
            Y2 = pers.tile([121, 17920], F32R, tag="Y2")
            STG = [pers.tile([NQ, 2048], F32, name=f"STG{i}", tag=f"STG{i}")
                   for i in range(2)]

            pstr = lambda t: t.ap[0][0]   # partition stride (elements)

            def y1slice(g, buf, cl, rows=121):
                base = (2 * g + buf) * CHUNK
                return Y1[0:rows, base:base + cl]

            def y2slice(g, buf, cl, rows=121):
                base = (5 * g + buf) * CHUNK
                return Y2[0:rows, base:base + cl]

            # ---------------- input DMAs ----------------
            nc.sync.dma_start(out=HSB[:, 0:300], in_=hsall[:, 0:300])
            nc.scalar.dma_start(out=WSB[:, 0:243], in_=wall[:, 0:243])
            nc.sync.dma_start(out=HSB[:, 300:700], in_=hsall[:, 300:700])
            nc.scalar.dma_start(out=WSB[:, 243:859], in_=wall[:, 243:859])

            # F layout: [0:8]=feat0, [8:10]=-gx,-gy, [10]=ones,
            #           [32:40]=feat1, [40:42]=-gx,-gy, [42]=ones,
            #           [64:72]=feat0, [72:74]=-gx,-gy, [74]=ones,
            #           [75:83]=feat1, [83:85]=-gx,-gy, [85]=ones
            nc.sync.dma_start(out=R(Ft[0:8, :]), in_=R(mf[0]))
            nc.sync.dma_start(out=R(Ft[8:11, :]), in_=R(cst[0:3, :]))
            nc.scalar.dma_start(out=R(Ft[32:40, :]), in_=R(mf[1]))
            nc.scalar.dma_start(out=R(Ft[40:43, :]), in_=R(cst[0:3, :]))
            # mixed band: SBUF->SBUF copies of the two main bands
            with tc.high_priority():
                nc.gpsimd.dma_start(out=R(Ft[64:75, :]), in_=R(Ft[0:11, :]))
                nc.gpsimd.dma_start(out=R(Ft[75:86, :]), in_=R(Ft[32:43, :]))
                # ones rows of Y1/Y2 (row 120) from the Ft ones row
                nc.gpsimd.dma_start(out=FV(Y1[120:121, :]),
                                    in_=FV(Ft[10:11, 0:7168]))
                nc.gpsimd.dma_start(out=FV(Y2[120:121, 0:8640]),
                                    in_=FV(Ft[10:11, 0:8640]))
                nc.gpsimd.dma_start(out=FV(Y2[120:121, 8640:17280]),
                                    in_=FV(Ft[10:11, 0:8640]))
                nc.gpsimd.dma_start(out=FV(Y2[120:121, 17280:17920]),
                                    in_=FV(Ft[10:11, 0:640]))

            # ---------------- zero fills ----------------
            nc.gpsimd.memset(FV(T2[:, :]), 0.0)
            nc.gpsimd.memset(FV(T3[:, :]), 0.0)
            nc.gpsimd.memset(FV(T1[64:86, 720:840]), 0.0)
            nc.gpsimd.memset(PW1S[0:9, 800:840], 0.0)
            nc.gpsimd.memset(PW23S[0:9, 100:105], 0.0)
            # mixed-group stale rows (80-119) of Y1/Y2 col blocks
            nc.gpsimd.memset(FV(Y1[64:120, 6144:7168]), 0.0)
            nc.gpsimd.memset(FV(Y2[64:120, 15360:17920]), 0.0)

            # ---------------- controller matmuls (fp16) ----------------
            with tc.tile_pool(name="psctrl", bufs=1, space="PSUM") as psc:
                pw1p = psc.tile([9, 1024], F32, tag="pw1p")
                pw23p = psc.tile([9, NQ], F32, tag="pw23p")
                pw0p = psc.tile([11, 1024], F32, tag="pw0p")
                # packed slot s holds logical chunk [0,1,6,2,3,4,5][s]
                # L2/L3 params contract over hs + tail chunks (slots 0,1,2)
                k3 = [(0, 128), (1, 128), (2, 3)]
                # L1 params (incl c row) contract over all 7 slots
                k7 = [(0, 128), (1, 128), (3, 128), (4, 128), (5, 128),
                      (6, 128), (2, 3)]
                # w1/b1 params first: T2 build is the long pole
                for i, (kc, kn) in enumerate(k3):
                    c0 = kc * 81
                    nc.tensor.matmul(
                        pw23p[0:9, 0:NQ],
                        WSB[0:kn, c0:c0 + 9],
                        HSB[0:kn, kc * 100:kc * 100 + NQ],
                        start=(i == 0), stop=(i == 2))
                for o2 in range(8):
                    for i, (kc, kn) in enumerate(k3):
                        c0 = kc * 81 + 9 + o2 * 9
                        nc.tensor.matmul(
                            pw1p[0:9, o2 * 128:o2 * 128 + NQ],
                            WSB[0:kn, c0:c0 + 9],
                            HSB[0:kn, kc * 100:kc * 100 + NQ],
                            start=(i == 0), stop=(i == 2))
                for o in range(8):
                    for i, (kc, kn) in enumerate(k7):
                        c0 = 243 + kc * 88 + o * 11
                        nc.tensor.matmul(
                            pw0p[0:11, o * 128:o * 128 + NQ],
                            WSB[0:kn, c0:c0 + 11],
                            HSB[0:kn, kc * 100:kc * 100 + NQ],
                            start=(i == 0), stop=(i == 6))

                # evacuate psum -> SBUF (the only way out of PSUM)
                nc.vector.tensor_copy(
                    PW1S[:, 0:400].rearrange("p (n o) -> p o n", o=8),
                    pw1p.rearrange("p (o n) -> p o n", o=8)[:, :, 0:50])
                nc.scalar.activation(
                    PW1S[:, 400:800].rearrange("p (n o) -> p o n", o=8),
                    pw1p.rearrange("p (o n) -> p o n", o=8)[:, :, 50:NQ],
                    Copy)
                nc.scalar.activation(PW23S[:, 0:NQ], pw23p[:, :], Copy)
                nc.scalar.activation(
                    PW0S[:, 0:400].rearrange("p (o n) -> p o n", o=4),
                    pw0p[:, 0:512].rearrange(
                        "p (o n) -> p o n", o=4)[:, :, 0:NQ], Copy)
                nc.vector.tensor_copy(
                    PW0S[:, 400:800].rearrange("p (o n) -> p o n", o=4),
                    pw0p[:, 512:1024].rearrange(
                        "p (o n) -> p o n", o=4)[:, :, 0:NQ])

            # ---------------- weight scatters ----------------
            # T2 per-j: dst [j*8:(j+1)*8, g*120+j*8+o2], src PW1S[o, (15g+j)*8+o2]
            dma_rot = [nc.sync, nc.sync, nc.sync, nc.gpsimd]
            for j in range(15):
                gcnt = 7 if j < 10 else 6
                if j % 4 == 0:
                    # 32-aligned partition base: engine copy
                    dst = T2[j * 8:j * 8 + 8, :].rearrange(
                        "p (g x) -> p g x", g=7)[:, 0:gcnt, j * 8:j * 8 + 8]
                    sv = PW1S[0:8, 0:840].rearrange(
                        "p (g x) -> p g x", g=7)[:, 0:gcnt, j * 8:j * 8 + 8]
                    if j % 8 == 0:
                        nc.vector.tensor_copy(dst, sv)
                    else:
                        nc.scalar.activation(dst, sv, Copy)
                else:
                    s = bass.AP(tensor=PW1S.tensor, offset=PW1S.offset + j * 8,
                                ap=[[pstr(PW1S), 8], [120, gcnt], [1, 8]])
                    dst = bass.AP(tensor=T2.tensor,
                                  offset=T2.offset + j * 8 * pstr(T2) + j * 8,
                                  ap=[[pstr(T2), 8], [120, gcnt], [1, 8]])
                    dma_rot[j % 4].dma_start(out=dst.bitcast(F32R),
                                             in_=s.bitcast(F32R))
            # T2 b1 row: dst [120, g*120+j*8+o2] <- PW1S[8, (15g+j)*8+o2]
            srcb1 = bass.AP(tensor=PW1S.tensor, offset=PW1S.offset + 8 * pstr(PW1S),
                            ap=[[pstr(PW1S), 1], [120, 7], [1, 120]])
            dstb1 = bass.AP(tensor=T2.tensor, offset=T2.offset + 120 * pstr(T2),
                            ap=[[pstr(T2), 1], [120, 7], [1, 120]])
            nc.sync.dma_start(out=dstb1.bitcast(F32R), in_=srcb1.bitcast(F32R))

            # T1 scatters (engine copies where partition base allows)
            # f0 block: rows 0-9 <- PW0S rows 0-9, cols (g,j,o) g-major
            src_f0 = bass.AP(tensor=PW0S.tensor, offset=PW0S.offset,
                             ap=[[pstr(PW0S), 11], [1, 45], [100, 8]])
            nc.vector.tensor_copy(T1[0:11, 0:360].rearrange(
                "p (n o) -> p n o", o=8), src_f0)
            src_f1 = bass.AP(tensor=PW0S.tensor, offset=PW0S.offset + 45,
                             ap=[[pstr(PW0S), 11], [1, 45], [100, 8]])
            nc.scalar.activation(T1[32:43, 360:720].rearrange(
                "p (n o) -> p n o", o=8), src_f1, Copy)
            # mixed f0-half rows 64-74 incl c (n' 90-94), base 64 ok
            src_m0 = bass.AP(tensor=PW0S.tensor, offset=PW0S.offset + 90,
                             ap=[[pstr(PW0S), 11], [1, 5], [100, 8]])
            nc.vector.tensor_copy(T1[64:75, 720:760].rearrange(
                "p (n o) -> p n o", o=8), src_m0)
            # mixed f1-half rows 75-85 incl c (n' 95-99): base 75 -> DMA/query
            for n in range(5):
                src_m1 = bass.AP(tensor=PW0S.tensor, offset=PW0S.offset + 95 + n,
                                 ap=[[pstr(PW0S), 11], [100, 8]])
                dst_m1 = bass.AP(tensor=T1.tensor,
                                 offset=T1.offset + 75 * pstr(T1) + 760 + n * 8,
                                 ap=[[pstr(T1), 11], [1, 8]])
                nc.sync.dma_start(out=dst_m1.bitcast(F32R),
                                  in_=src_m1.bitcast(F32R))

            # T3 per-j: dst [j*8+o rows, col 115g+j], src PW23S[o, 15g+j]
            dma_rot3 = [nc.sync, nc.gpsimd]
            for j in range(15):
                gcnt = 7 if j < 10 else 6
                s = bass.AP(tensor=PW23S.tensor, offset=PW23S.offset + j,
                            ap=[[pstr(PW23S), 8], [15, gcnt], [1, 1]])
                dst = bass.AP(tensor=T3.tensor,
                              offset=T3.offset + j * 8 * pstr(T3) + j,
                              ap=[[pstr(T3), 8], [115, gcnt], [1, 1]])
                dma_rot3[j % 2].dma_start(out=dst.bitcast(F32R),
                                          in_=s.bitcast(F32R))
            # T3 b2 row: dst [120, 115g+j], padded to 7 groups x 15
            srcb2a = bass.AP(tensor=PW23S.tensor,
                             offset=PW23S.offset + 8 * pstr(PW23S),
                             ap=[[pstr(PW23S), 1], [15, 7], [1, 15]])
            dstb2a = bass.AP(tensor=T3.tensor, offset=T3.offset + 120 * pstr(T3),
                             ap=[[pstr(T3), 1], [115, 7], [1, 15]])
            nc.gpsimd.dma_start(out=dstb2a.bitcast(F32R), in_=srcb2a.bitcast(F32R))

            # ---------------- main loop (skewed) ----------------
            psm_cm = tc.tile_pool(name="psmain", bufs=1, space="PSUM")
            psm = psm_cm.__enter__()
            ps1 = [psm.tile([120, CHUNK], F32, name=f"ps1_{i}", tag=f"ps1_{i}")
                   for i in range(3)]
            ps2 = [psm.tile([120, CHUNK], F32, name=f"ps2_{i}", tag=f"ps2_{i}")
                   for i in range(3)]
            ps3 = [psm.tile([NQ, CHUNK], F32, name=f"ps3_{i}", tag=f"ps3_{i}")
                   for i in range(2)]

            out_q = [nc.sync, nc.gpsimd]

            def emit_L3(cp):
                coff, clen = CHUNKS[cp]
                ph = ps3[cp % 2]
                for g in range(7):
                    nc.tensor.matmul(
                        ph[0:NQ, 0:clen],
                        T3[0:121, 100 * g:100 * g + NQ],
                        y2slice(g, cp % 5, clen),
                        start=(g == 0), stop=(g == 6),
                        skip_group_check=True)

            def emit_L3copy(cp):
                coff, clen = CHUNKS[cp]
                ph = ps3[cp % 2]
                qi = next(i for i, (a, b) in enumerate(QUARTERS) if a <= cp < b)
                q0 = CHUNKS[QUARTERS[qi][0]][0]
                stg = STG[qi % 2]
                nc.scalar.activation(stg[0:NQ, coff - q0:coff - q0 + clen],
                                     ph[0:NQ, 0:clen], Copy)
                if cp == QUARTERS[qi][1] - 1:
                    qlen = coff + clen - q0
                    last = (qi == len(QUARTERS) - 1)
                    qa = out_q[qi % 2]
                    qb = out_q[(qi + 1) % 2]
                    qc = nc.scalar if last else out_q[qi % 2]
                    qa.dma_start(out=outp[0:45, q0:q0 + qlen],
                                 in_=stg[0:45, 0:qlen])
                    qb.dma_start(out=outp[50:95, q0:q0 + qlen],
                                 in_=stg[45:90, 0:qlen])
                    qc.dma_start(out=outp[45:50, q0:q0 + qlen],
                                 in_=stg[90:95, 0:qlen])
                    qc.dma_start(out=outp[95:100, q0:q0 + qlen],
                                 in_=stg[95:100, 0:qlen])

            def emit_L1(ci, g):
                coff, clen = CHUNKS[ci]
                band, k1, nq = GROUPS[g]
                m = nq * 8
                pa = ps1[(ci * 7 + g) % 3]
                nc.tensor.matmul(
                    pa[0:m, 0:clen],
                    T1[band:band + k1, g * 120:g * 120 + m],
                    Ft[band:band + k1, coff:coff + clen],
                    start=True, stop=True)
                relu = relu_act if g % 2 == 0 else relu_dve
                relu(y1slice(g, ci % 2, clen, rows=m), pa[0:m, 0:clen])

            def emit_L2(ci, g):
                coff, clen = CHUNKS[ci]
                _, _, nq = GROUPS[g]
                m = nq * 8
                pb = ps2[(ci * 7 + g) % 3]
                nc.tensor.matmul(
                    pb[0:m, 0:clen],
                    T2[0:121, g * 120:g * 120 + m],
                    y1slice(g, ci % 2, clen),
                    start=True, stop=True)
                relu = relu_dve if g % 2 == 0 else relu_act
                relu(y2slice(g, ci % 5, clen, rows=m), pb[0:m, 0:clen])

            l3_next = [0]
            for ci in range(NCH + 2):
                pend = []
                if ci < NCH:
                    for g in range(3):
                        emit_L1(ci, g)
                if ci >= 3:
                    # catch-up: 2 chains while behind, and again at the tail
                    n_emit = 2 if (l3_next[0] < ci - 3 or ci >= NCH - 1) else 1
                    limit = ci - 1 if ci >= NCH - 1 else ci - 2
                    for _ in range(n_emit):
                        if l3_next[0] <= min(limit, NCH - 1):
                            emit_L3(l3_next[0])
                            pend.append(l3_next[0])
                            l3_next[0] += 1
                if ci < NCH:
                    for g in range(7):
                        emit_L2(ci, g)
                        if g + 3 < 7:
                            emit_L1(ci, g + 3)
                        if g == 0:
                            for cp in pend:
                                emit_L3copy(cp)
                            pend = []
                for cp in pend:
                    emit_L3copy(cp)
            while l3_next[0] < NCH:
                emit_L3(l3_next[0])
                emit_L3copy(l3_next[0])
                l3_next[0] += 1
            psm_cm.__exit__(None, None, None)

    nc.compile()
    return nc


_NC = None


def _get_nc():
    global _NC
    if _NC is None:
        _NC = _build_program()
    return _NC


# n' permutation: group-major query order per core
NPERM = np.concatenate([np.arange(0, 45), np.arange(50, 95),
                        np.arange(45, 50), np.arange(95, 100)])


def _host_pack(hs, mask_features, references, sizes, W_ctrl, b_ctrl):
    hs = np.asarray(hs, np.float32)
    mask_features = np.asarray(mask_features, np.float32)
    references = np.asarray(references, np.float32)
    sizes = np.asarray(sizes, np.float32)
    W_ctrl = np.asarray(W_ctrl, np.float32)
    b_ctrl = np.asarray(b_ctrl, np.float32)

    # pixel grid
    xs = np.arange(W, dtype=np.float32) * STRIDE + STRIDE // 2
    ys = np.arange(H, dtype=np.float32) * STRIDE + STRIDE // 2
    gxf = np.tile(xs, H)
    gyf = np.repeat(ys, W)
    cstm = np.stack([-gxf, -gyf, np.ones(P, np.float32)]).astype(np.float32)

    # wall [771, 169]: rows contract against [hs; hs*rx; hs*ry; 1; rx; ry]
    W_aug = np.concatenate([W_ctrl.T, b_ctrl[None, :]], 0)  # [257, 169]
    wall = np.zeros((771, 169), np.float32)

    def setcol(c, p):
        wall[0:256, c] = W_aug[0:256, p]
        wall[768, c] = W_aug[256, p]

    for o in range(8):
        for i in range(10):
            setcol(o * 11 + i, o * 10 + i)
        c = o * 11 + 10                      # c = w8*rx + w9*ry + b0
        wall[256:512, c] = W_aug[0:256, o * 10 + 8]
        wall[769, c] = W_aug[256, o * 10 + 8]
        wall[512:768, c] = W_aug[0:256, o * 10 + 9]
        wall[770, c] = W_aug[256, o * 10 + 9]
        wall[0:256, c] = W_aug[0:256, 152 + o]
        wall[768, c] = W_aug[256, 152 + o]
    for o2 in range(8):
        for j in range(8):
            setcol(88 + o2 * 9 + j, 80 + o2 * 8 + j)
        setcol(88 + o2 * 9 + 8, 160 + o2)
    for j in range(8):
        setcol(160 + j, 144 + j)
    setcol(168, 168)
    # packed slot s holds logical k-chunk [0, 1, 6(tail), 2, 3, 4, 5][s]:
    # first 3x81 = w2/b2 + w1/b1 cols of slots 0-2, then 7x88 w0 cols
    wpk = np.zeros((128, 859), np.float32)
    for s, kc in enumerate([0, 1, None, 2, 3, 4, 5]):
        rows = wall[768:771] if kc is None else wall[128 * kc:128 * (kc + 1)]
        nr = rows.shape[0]
        if s < 3:
            wpk[0:nr, s * 81:s * 81 + 9] = rows[:, 160:169]
            wpk[0:nr, s * 81 + 9:s * 81 + 81] = rows[:, 88:160]
        wpk[0:nr, 243 + s * 88:243 + s * 88 + 88] = rows[:, 0:88]
    wall = wpk.astype(np.float16)

    # reference points in pixels
    b_idx = np.arange(16) // 8
    scale = sizes[b_idx][:, ::-1]                      # [16, 2] = (img_w, img_h)
    refs_px = references * scale[:, None, :]           # [16, 50, 2]

    in_maps = []
    for c in range(NCORES):
        hs_c = hs[2 * c:2 * c + 2].reshape(NQ, HID)[NPERM]
        mf_c = mask_features[2 * c:2 * c + 2].reshape(2, 8, P)
        rp = refs_px[2 * c:2 * c + 2].reshape(NQ, 2)[NPERM]
        hrows = np.empty((771, NQ), np.float32)
        hrows[0:256] = hs_c.T
        hrows[256:512] = hs_c.T * rp[:, 0][None, :]
        hrows[512:768] = hs_c.T * rp[:, 1][None, :]
        hrows[768] = 1.0
        hrows[769] = rp[:, 0]
        hrows[770] = rp[:, 1]
        hpk = np.zeros((128, 700), np.float32)
        for s, kc in enumerate([0, 1, None, 2, 3, 4, 5]):
            if kc is None:
                hpk[0:3, s * 100:s * 100 + 100] = hrows[768:771]
            else:
                hpk[:, s * 100:s * 100 + 100] = hrows[128 * kc:128 * (kc + 1)]
        in_maps.append(dict(
            mf=np.ascontiguousarray(mf_c),
            hsall=hpk.astype(np.float16),
            wall=wall,
            cst=cstm,
        ))
    return in_maps


def kernel(hs, mask_features, references, sizes, W_ctrl, b_ctrl, T):
    assert int(T) == 8
    nc = _get_nc()
    in_maps = _host_pack(hs, mask_features, references, sizes, W_ctrl, b_ctrl)
    res = bass_utils.run_bass_kernel_spmd(nc, in_maps, core_ids=list(range(NCORES)))
    out = np.empty((16, Q, H, W), np.float32)
    for c in range(NCORES):
        out[2 * c:2 * c + 2] = res.results[c]["outp"].reshape(2, Q, H, W)
    return out


# revision 13
# speedup vs baseline: 1.0696x; 1.0178x over previous
"""DynamicSegmentationHead Trainium2 kernel, restructured for overlap.

Data-parallel over 16 clip-frames: each core handles 2 frames (100 queries).
Query order is host-permuted to group-major n' = [f0 q0-44, f1 q0-44,
f0 q45-49, f1 q45-49] so every weight-scatter is one uniform strided DMA
and the L3 psum block maps to output rows with 3 clean DMAs per flush.

Controller runs in fp16 over an extended K of 771: host supplies
[hs; hs*refx; hs*refy; ones; refx; refy] so the per-query constant row
c = w8*refx + w9*refy + b0 falls out of the matmul directly (no separate
c-row pipeline). pw23 -> pw1 -> pw0 ordering releases the T3/T2/T1 build
paths in deadline order.

Per core, per 512-pixel chunk (17 chunks):
  L1: y1 = relu(T1 @ Ft)    7 matmuls K=11/22, M=120/80, psum rot x3
  L2: y2 = relu(T2 @ y1)    7 matmuls K=121 blockdiag, psum rot x3
  L3: one [100,512] psum x2, 7 accumulating matmuls, emitted with a
      progressive 2-chunk skew (first chains deferred to iteration 3 so
      they never wait on the T3 scatter DMAs), Y2 5-deep
  PSUM evacuations of the controller params split across ACT+DVE halves
  ACT stage copy + output flush DMAs per span, relus split ACT/DVE.

Steady state is ACT/DVE-bound: 15 psum-evacuation ops per chunk across
2 engines is the structural floor (PSUM has 8 banks; DMA cannot read
PSUM; 7 query groups is minimal since 800 M-rows / 128 partitions).
"""

import numpy as np

import concourse.bass as bass
import concourse.bacc as bacc
import concourse.tile as tile
from concourse import mybir
from concourse import bass_utils

F32 = mybir.dt.float32
F32R = mybir.dt.float32r
F16 = mybir.dt.float16

HID = 256
Q = 50
H, W = 72, 120
P = H * W            # 8640
NQ = 100             # queries per core (2 frames)
NCORES = 8
STRIDE = 4

CHUNK = 512
NCH = 17
CHUNKS = [(i * CHUNK, CHUNK) for i in range(16)] + [(16 * CHUNK, P - 16 * CHUNK)]
QUARTERS = [(0, 4), (4, 8), (8, 12), (12, 16), (16, 17)]   # flush spans

# groups in n' (group-major) order: 3 f0, 3 f1, 1 mixed
# (band, k1, nq); n'-base of group g is 15*g
GROUPS = [(0, 11, 15)] * 3 + [(32, 11, 15)] * 3 + [(64, 22, 10)]

Relu = mybir.ActivationFunctionType.Relu
Copy = mybir.ActivationFunctionType.Copy


def _build_program():
    nc = bacc.Bacc("TRN2", target_bir_lowering=False, debug=False)
    R = lambda ap: ap.bitcast(F32R)
    FV = lambda ap: ap.bitcast(F32)

    mf = nc.dram_tensor("mf", [2, 8, P], F32, kind="ExternalInput").ap()
    # hsall: 7 k-chunks of [hs(256); hs*rx(256); hs*ry(256); 1; rx; ry],
    # packed column-major in chunk order [0, 1, 6, 2, 3, 4, 5] so the
    # L2/L3-critical chunks (0, 1, 6) transfer first.
    hsall = nc.dram_tensor("hsall", [128, 700], F16, kind="ExternalInput").ap()
    # wall: [3 x 81 w23+w1 cols (slots 0-2)] then [7 x 88 w0 cols]
    wall = nc.dram_tensor("wall", [128, 859], F16, kind="ExternalInput").ap()
    cst = nc.dram_tensor("cst", [3, P], F32, kind="ExternalInput").ap()
    outp = nc.dram_tensor("outp", [NQ, P], F32, kind="ExternalOutput").ap()

    def relu_act(out_ap, in_ap):
        nc.scalar.activation(out_ap, in_ap, Relu)

    def relu_dve(out_ap, in_ap):
        nc.vector.tensor_scalar(out=out_ap, in0=in_ap, scalar1=0.0,
                                scalar2=None, op0=mybir.AluOpType.max)

    with tile.TileContext(nc) as tc:
        with tc.tile_pool(name="persist", bufs=1) as pers:
            # ---------------- persistent SBUF ----------------
            Ft = pers.tile([128, P], F32R, tag="F")
            HSB = pers.tile([128, 700], F16, tag="HSB")
            WSB = pers.tile([128, 859], F16, tag="WSB")
            PW0S = pers.tile([11, 800], F32, tag="PW0S")
            PW1S = pers.tile([9, 840], F32, tag="PW1S")
            PW23S = pers.tile([9, 105], F32, tag="PW23S")
            wsrc = pers.tile([1, 512], F32, tag="wsrc")

            T1 = pers.tile([86, 840], F32R, tag="T1")
            T2 = pers.tile([121, 840], F32R, tag="T2")
            T3 = pers.tile([121, 705], F32R, tag="T3")
            Y1 = pers.tile([121, 7168], F32R, tag="Y1")   # (g,buf) col blocks
            Y2 = pers.tile([121, 17920], F32R, tag="Y2")
            STG = [pers.tile([NQ, 2048], F32, name=f"STG{i}", tag=f"STG{i}")
                   for i in range(2)]

            pstr = lambda t: t.ap[0][0]   # partition stride (elements)

            def y1slice(g, buf, cl, rows=121):
                base = (2 * g + buf) * CHUNK
                return Y1[0:rows, base:base + cl]

            def y2slice(g, buf, cl, rows=121):
                base = (5 * g + buf) * CHUNK
                return Y2[0:rows, base:base + cl]

            # ---------------- input DMAs ----------------
            nc.sync.dma_start(out=HSB[:, 0:300], in_=hsall[:, 0:300])
            nc.scalar.dma_start(out=WSB[:, 0:243], in_=wall[:, 0:243])
            nc.sync.dma_start(out=HSB[:, 300:700], in_=hsall[:, 300:700])
            nc.scalar.dma_start(out=WSB[:, 243:859], in_=wall[:, 243:859])

            # F layout: [0:8]=feat0, [8:10]=-gx,-gy, [10]=ones,
            #           [32:40]=feat1, [40:42]=-gx,-gy, [42]=ones,
            #           [64:72]=feat0, [72:74]=-gx,-gy, [74]=ones,
            #           [75:83]=feat1, [83:85]=-gx,-gy, [85]=ones
            nc.sync.dma_start(out=R(Ft[0:8, :]), in_=R(mf[0]))
            nc.sync.dma_start(out=R(Ft[8:11, :]), in_=R(cst[0:3, :]))
            nc.scalar.dma_start(out=R(Ft[32:40, :]), in_=R(mf[1]))
            nc.scalar.dma_start(out=R(Ft[40:43, :]), in_=R(cst[0:3, :]))
            # mixed band: SBUF->SBUF copies of the two main bands
            with tc.high_priority():
                nc.gpsimd.dma_start(out=R(Ft[64:75, :]), in_=R(Ft[0:11, :]))
                nc.gpsimd.dma_start(out=R(Ft[75:86, :]), in_=R(Ft[32:43, :]))
                # ones rows of Y1/Y2 (row 120) from the Ft ones row
                nc.gpsimd.dma_start(out=FV(Y1[120:121, :]),
                                    in_=FV(Ft[10:11, 0:7168]))
                nc.gpsimd.dma_start(out=FV(Y2[120:121, 0:8640]),
                                    in_=FV(Ft[10:11, 0:8640]))
                nc.gpsimd.dma_start(out=FV(Y2[120:121, 8640:17280]),
                                    in_=FV(Ft[10:11, 0:8640]))
                nc.gpsimd.dma_start(out=FV(Y2[120:121, 17280:17920]),
                                    in_=FV(Ft[10:11, 0:640]))

            # PE warm-up source (see warm-up matmuls below)
            nc.vector.memset(wsrc[:, :], 0.0)

            # ---------------- zero fills ----------------
            nc.gpsimd.memset(FV(T2[:, :]), 0.0)
            nc.gpsimd.memset(FV(T3[:, :]), 0.0)
            nc.gpsimd.memset(FV(T1[64:86, 720:840]), 0.0)
            nc.gpsimd.memset(PW1S[0:9, 800:840], 0.0)
            nc.gpsimd.memset(PW23S[0:9, 100:105], 0.0)
            # mixed-group stale rows (80-119) of Y1/Y2 col blocks
            nc.gpsimd.memset(FV(Y1[64:120, 6144:7168]), 0.0)
            nc.gpsimd.memset(FV(Y2[64:120, 15360:17920]), 0.0)

            # ---------------- controller matmuls (fp16) ----------------
            with tc.tile_pool(name="psctrl", bufs=1, space="PSUM") as psc:
                # p-state warm-up: dependency-free matmuls keep the PE busy
                # from t~0 so the controller runs at full clock. Results are
                # never read.
                wps = [psc.tile([1, 512], F32, name=f"wps{i}", tag=f"wps{i}")
                       for i in range(2)]
                for i in range(5):
                    nc.tensor.matmul(wps[i % 2][0:1, 0:512],
                                     R(wsrc[0:1, 0:1]), R(wsrc[0:1, 0:512]),
                                     start=True, stop=True)
                pw1p = psc.tile([9, 1024], F32, tag="pw1p")
                pw23p = psc.tile([9, NQ], F32, tag="pw23p")
                pw0p = psc.tile([11, 1024], F32, tag="pw0p")
                # packed slot s holds logical chunk [0,1,6,2,3,4,5][s]
                # L2/L3 params contract over hs + tail chunks (slots 0,1,2)
                k3 = [(0, 128), (1, 128), (2, 3)]
                # L1 params (incl c row) contract over all 7 slots
                k7 = [(0, 128), (1, 128), (3, 128), (4, 128), (5, 128),
                      (6, 128), (2, 3)]
                # w1/b1 params first: T2 build is the long pole
                for i, (kc, kn) in enumerate(k3):
                    c0 = kc * 81
                    nc.tensor.matmul(
                        pw23p[0:9, 0:NQ],
                        WSB[0:kn, c0:c0 + 9],
                        HSB[0:kn, kc * 100:kc * 100 + NQ],
                        start=(i == 0), stop=(i == 2))
                for o2 in range(8):
                    for i, (kc, kn) in enumerate(k3):
                        c0 = kc * 81 + 9 + o2 * 9
                        nc.tensor.matmul(
                            pw1p[0:9, o2 * 128:o2 * 128 + NQ],
                            WSB[0:kn, c0:c0 + 9],
                            HSB[0:kn, kc * 100:kc * 100 + NQ],
                            start=(i == 0), stop=(i == 2))
                for o in range(8):
                    for i, (kc, kn) in enumerate(k7):
                        c0 = 243 + kc * 88 + o * 11
                        nc.tensor.matmul(
                            pw0p[0:11, o * 128:o * 128 + NQ],
                            WSB[0:kn, c0:c0 + 11],
                            HSB[0:kn, kc * 100:kc * 100 + NQ],
                            start=(i == 0), stop=(i == 6))

                # evacuate psum -> SBUF (the only way out of PSUM)
                nc.vector.tensor_copy(
                    PW1S[:, 0:400].rearrange("p (n o) -> p o n", o=8),
                    pw1p.rearrange("p (o n) -> p o n", o=8)[:, :, 0:50])
                nc.scalar.activation(
                    PW1S[:, 400:800].rearrange("p (n o) -> p o n", o=8),
                    pw1p.rearrange("p (o n) -> p o n", o=8)[:, :, 50:NQ],
                    Copy)
                nc.scalar.activation(PW23S[:, 0:NQ], pw23p[:, :], Copy)
                nc.scalar.activation(
                    PW0S[:, 0:400].rearrange("p (o n) -> p o n", o=4),
                    pw0p[:, 0:512].rearrange(
                        "p (o n) -> p o n", o=4)[:, :, 0:NQ], Copy)
                nc.vector.tensor_copy(
                    PW0S[:, 400:800].rearrange("p (o n) -> p o n", o=4),
                    pw0p[:, 512:1024].rearrange(
                        "p (o n) -> p o n", o=4)[:, :, 0:NQ])

            # ---------------- weight scatters ----------------
            # T2 per-j: dst [j*8:(j+1)*8, g*120+j*8+o2], src PW1S[o, (15g+j)*8+o2]
            dma_rot = [nc.sync, nc.sync, nc.sync, nc.gpsimd]
            for j in range(15):
                gcnt = 7 if j < 10 else 6
                if j % 4 == 0:
                    # 32-aligned partition base: engine copy
                    dst = T2[j * 8:j * 8 + 8, :].rearrange(
                        "p (g x) -> p g x", g=7)[:, 0:gcnt, j * 8:j * 8 + 8]
                    sv = PW1S[0:8, 0:840].rearrange(
                        "p (g x) -> p g x", g=7)[:, 0:gcnt, j * 8:j * 8 + 8]
                    if j % 8 == 0:
                        nc.vector.tensor_copy(dst, sv)
                    else:
                        nc.scalar.activation(dst, sv, Copy)
                else:
                    s = bass.AP(tensor=PW1S.tensor, offset=PW1S.offset + j * 8,
                                ap=[[pstr(PW1S), 8], [120, gcnt], [1, 8]])
                    dst = bass.AP(tensor=T2.tensor,
                                  offset=T2.offset + j * 8 * pstr(T2) + j * 8,
                                  ap=[[pstr(T2), 8], [120, gcnt], [1, 8]])
                    dma_rot[j % 4].dma_start(out=dst.bitcast(F32R),
                                             in_=s.bitcast(F32R))
            # T2 b1 row: dst [120, g*120+j*8+o2] <- PW1S[8, (15g+j)*8+o2]
            srcb1 = bass.AP(tensor=PW1S.tensor, offset=PW1S.offset + 8 * pstr(PW1S),
                            ap=[[pstr(PW1S), 1], [120, 7], [1, 120]])
            dstb1 = bass.AP(tensor=T2.tensor, offset=T2.offset + 120 * pstr(T2),
                            ap=[[pstr(T2), 1], [120, 7], [1, 120]])
            nc.sync.dma_start(out=dstb1.bitcast(F32R), in_=srcb1.bitcast(F32R))

            # T1 scatters (engine copies where partition base allows)
            # f0 block: rows 0-9 <- PW0S rows 0-9, cols (g,j,o) g-major
            src_f0 = bass.AP(tensor=PW0S.tensor, offset=PW0S.offset,
                             ap=[[pstr(PW0S), 11], [1, 45], [100, 8]])
            nc.vector.tensor_copy(T1[0:11, 0:360].rearrange(
                "p (n o) -> p n o", o=8), src_f0)
            src_f1 = bass.AP(tensor=PW0S.tensor, offset=PW0S.offset + 45,
                             ap=[[pstr(PW0S), 11], [1, 45], [100, 8]])
            nc.scalar.activation(T1[32:43, 360:720].rearrange(
                "p (n o) -> p n o", o=8), src_f1, Copy)
            # mixed f0-half rows 64-74 incl c (n' 90-94), base 64 ok
            src_m0 = bass.AP(tensor=PW0S.tensor, offset=PW0S.offset + 90,
                             ap=[[pstr(PW0S), 11], [1, 5], [100, 8]])
            nc.vector.tensor_copy(T1[64:75, 720:760].rearrange(
                "p (n o) -> p n o", o=8), src_m0)
            # mixed f1-half rows 75-85 incl c (n' 95-99): base 75 -> DMA/query
            for n in range(5):
                src_m1 = bass.AP(tensor=PW0S.tensor, offset=PW0S.offset + 95 + n,
                                 ap=[[pstr(PW0S), 11], [100, 8]])
                dst_m1 = bass.AP(tensor=T1.tensor,
                                 offset=T1.offset + 75 * pstr(T1) + 760 + n * 8,
                                 ap=[[pstr(T1), 11], [1, 8]])
                nc.sync.dma_start(out=dst_m1.bitcast(F32R),
                                  in_=src_m1.bitcast(F32R))

            # T3 per-j: dst [j*8+o rows, col 115g+j], src PW23S[o, 15g+j]
            dma_rot3 = [nc.sync, nc.gpsimd]
            for j in range(15):
                gcnt = 7 if j < 10 else 6
                s = bass.AP(tensor=PW23S.tensor, offset=PW23S.offset + j,
                            ap=[[pstr(PW23S), 8], [15, gcnt], [1, 1]])
                dst = bass.AP(tensor=T3.tensor,
                              offset=T3.offset + j * 8 * pstr(T3) + j,
                              ap=[[pstr(T3), 8], [115, gcnt], [1, 1]])
                dma_rot3[j % 2].dma_start(out=dst.bitcast(F32R),
                                          in_=s.bitcast(F32R))
            # T3 b2 row: dst [120, 115g+j], padded to 7 groups x 15
            srcb2a = bass.AP(tensor=PW23S.tensor,
                             offset=PW23S.offset + 8 * pstr(PW23S),
                             ap=[[pstr(PW23S), 1], [15, 7], [1, 15]])
            dstb2a = bass.AP(tensor=T3.tensor, offset=T3.offset + 120 * pstr(T3),
                             ap=[[pstr(T3), 1], [115, 7], [1, 15]])
            nc.gpsimd.dma_start(out=dstb2a.bitcast(F32R), in_=srcb2a.bitcast(F32R))

            # ---------------- main loop (skewed) ----------------
            psm_cm = tc.tile_pool(name="psmain", bufs=1, space="PSUM")
            psm = psm_cm.__enter__()
            ps1 = [psm.tile([120, CHUNK], F32, name=f"ps1_{i}", tag=f"ps1_{i}")
                   for i in range(3)]
            ps2 = [psm.tile([120, CHUNK], F32, name=f"ps2_{i}", tag=f"ps2_{i}")
                   for i in range(3)]
            ps3 = [psm.tile([NQ, CHUNK], F32, name=f"ps3_{i}", tag=f"ps3_{i}")
                   for i in range(2)]

            out_q = [nc.sync, nc.gpsimd]

            def emit_L3(cp):
                coff, clen = CHUNKS[cp]
                ph = ps3[cp % 2]
                for g in range(7):
                    nc.tensor.matmul(
                        ph[0:NQ, 0:clen],
                        T3[0:121, 100 * g:100 * g + NQ],
                        y2slice(g, cp % 5, clen),
                        start=(g == 0), stop=(g == 6),
                        skip_group_check=True)

            def emit_L3copy(cp):
                coff, clen = CHUNKS[cp]
                ph = ps3[cp % 2]
                qi = next(i for i, (a, b) in enumerate(QUARTERS) if a <= cp < b)
                q0 = CHUNKS[QUARTERS[qi][0]][0]
                stg = STG[qi % 2]
                nc.scalar.activation(stg[0:NQ, coff - q0:coff - q0 + clen],
                                     ph[0:NQ, 0:clen], Copy)
                if cp == QUARTERS[qi][1] - 1:
                    qlen = coff + clen - q0
                    last = (qi == len(QUARTERS) - 1)
                    qa = out_q[qi % 2]
                    qb = out_q[(qi + 1) % 2]
                    qc = nc.scalar if last else out_q[qi % 2]
                    qa.dma_start(out=outp[0:45, q0:q0 + qlen],
                                 in_=stg[0:45, 0:qlen])
                    qb.dma_start(out=outp[50:95, q0:q0 + qlen],
                                 in_=stg[45:90, 0:qlen])
                    qc.dma_start(out=outp[45:50, q0:q0 + qlen],
                                 in_=stg[90:95, 0:qlen])
                    qc.dma_start(out=outp[95:100, q0:q0 + qlen],
                                 in_=stg[95:100, 0:qlen])

            def emit_L1(ci, g):
                coff, clen = CHUNKS[ci]
                band, k1, nq = GROUPS[g]
                m = nq * 8
                pa = ps1[(ci * 7 + g) % 3]
                nc.tensor.matmul(
                    pa[0:m, 0:clen],
                    T1[band:band + k1, g * 120:g * 120 + m],
                    Ft[band:band + k1, coff:coff + clen],
                    start=True, stop=True)
                relu = relu_act if g % 2 == 0 else relu_dve
                relu(y1slice(g, ci % 2, clen, rows=m), pa[0:m, 0:clen])

            def emit_L2(ci, g):
                coff, clen = CHUNKS[ci]
                _, _, nq = GROUPS[g]
                m = nq * 8
                pb = ps2[(ci * 7 + g) % 3]
                nc.tensor.matmul(
                    pb[0:m, 0:clen],
                    T2[0:121, g * 120:g * 120 + m],
                    y1slice(g, ci % 2, clen),
                    start=True, stop=True)
                relu = relu_dve if g % 2 == 0 else relu_act
                relu(y2slice(g, ci % 5, clen, rows=m), pb[0:m, 0:clen])

            l3_next = [0]
            for ci in range(NCH + 2):
                pend = []
                if ci < NCH:
                    for g in range(3):
                        emit_L1(ci, g)
                if ci >= 3:
                    # catch-up: 2 chains while behind, and again at the tail
                    n_emit = 2 if (l3_next[0] < ci - 3 or ci >= NCH - 1) else 1
                    limit = ci - 1 if ci >= NCH - 1 else ci - 2
                    for _ in range(n_emit):
                        if l3_next[0] <= min(limit, NCH - 1):
                            emit_L3(l3_next[0])
                            pend.append(l3_next[0])
                            l3_next[0] += 1
                if ci < NCH:
                    for g in range(7):
                        emit_L2(ci, g)
                        if g + 3 < 7:
                            emit_L1(ci, g + 3)
                        if g == 0:
                            for cp in pend:
                                emit_L3copy(cp)
                            pend = []
                for cp in pend:
                    emit_L3copy(cp)
            while l3_next[0] < NCH:
                emit_L3(l3_next[0])
                emit_L3copy(l3_next[0])
                l3_next[0] += 1
            psm_cm.__exit__(None, None, None)

    nc.compile()
    return nc


_NC = None


def _get_nc():
    global _NC
    if _NC is None:
        _NC = _build_program()
    return _NC


# n' permutation: group-major query order per core
NPERM = np.concatenate([np.arange(0, 45), np.arange(50, 95),
                        np.arange(45, 50), np.arange(95, 100)])


def _host_pack(hs, mask_features, references, sizes, W_ctrl, b_ctrl):
    hs = np.asarray(hs, np.float32)
    mask_features = np.asarray(mask_features, np.float32)
    references = np.asarray(references, np.float32)
    sizes = np.asarray(sizes, np.float32)
    W_ctrl = np.asarray(W_ctrl, np.float32)
    b_ctrl = np.asarray(b_ctrl, np.float32)

    # pixel grid
    xs = np.arange(W, dtype=np.float32) * STRIDE + STRIDE // 2
    ys = np.arange(H, dtype=np.float32) * STRIDE + STRIDE // 2
    gxf = np.tile(xs, H)
    gyf = np.repeat(ys, W)
    cstm = np.stack([-gxf, -gyf, np.ones(P, np.float32)]).astype(np.float32)

    # wall [771, 169]: rows contract against [hs; hs*rx; hs*ry; 1; rx; ry]
    W_aug = np.concatenate([W_ctrl.T, b_ctrl[None, :]], 0)  # [257, 169]
    wall = np.zeros((771, 169), np.float32)

    def setcol(c, p):
        wall[0:256, c] = W_aug[0:256, p]
        wall[768, c] = W_aug[256, p]

    for o in range(8):
        for i in range(10):
            setcol(o * 11 + i, o * 10 + i)
        c = o * 11 + 10                      # c = w8*rx + w9*ry + b0
        wall[256:512, c] = W_aug[0:256, o * 10 + 8]
        wall[769, c] = W_aug[256, o * 10 + 8]
        wall[512:768, c] = W_aug[0:256, o * 10 + 9]
        wall[770, c] = W_aug[256, o * 10 + 9]
        wall[0:256, c] = W_aug[0:256, 152 + o]
        wall[768, c] = W_aug[256, 152 + o]
    for o2 in range(8):
        for j in range(8):
            setcol(88 + o2 * 9 + j, 80 + o2 * 8 + j)
        setcol(88 + o2 * 9 + 8, 160 + o2)
    for j in range(8):
        setcol(160 + j, 144 + j)
    setcol(168, 168)
    # packed slot s holds logical k-chunk [0, 1, 6(tail), 2, 3, 4, 5][s]:
    # first 3x81 = w2/b2 + w1/b1 cols of slots 0-2, then 7x88 w0 cols
    wpk = np.zeros((128, 859), np.float32)
    for s, kc in enumerate([0, 1, None, 2, 3, 4, 5]):
        rows = wall[768:771] if kc is None else wall[128 * kc:128 * (kc + 1)]
        nr = rows.shape[0]
        if s < 3:
            wpk[0:nr, s * 81:s * 81 + 9] = rows[:, 160:169]
            wpk[0:nr, s * 81 + 9:s * 81 + 81] = rows[:, 88:160]
        wpk[0:nr, 243 + s * 88:243 + s * 88 + 88] = rows[:, 0:88]
    wall = wpk.astype(np.float16)

    # reference points in pixels
    b_idx = np.arange(16) // 8
    scale = sizes[b_idx][:, ::-1]                      # [16, 2] = (img_w, img_h)
    refs_px = references * scale[:, None, :]           # [16, 50, 2]

    in_maps = []
    for c in range(NCORES):
        hs_c = hs[2 * c:2 * c + 2].reshape(NQ, HID)[NPERM]
        mf_c = mask_features[2 * c:2 * c + 2].reshape(2, 8, P)
        rp = refs_px[2 * c:2 * c + 2].reshape(NQ, 2)[NPERM]
        hrows = np.empty((771, NQ), np.float32)
        hrows[0:256] = hs_c.T
        hrows[256:512] = hs_c.T * rp[:, 0][None, :]
        hrows[512:768] = hs_c.T * rp[:, 1][None, :]
        hrows[768] = 1.0
        hrows[769] = rp[:, 0]
        hrows[770] = rp[:, 1]
        hpk = np.zeros((128, 700), np.float32)
        for s, kc in enumerate([0, 1, None, 2, 3, 4, 5]):
            if kc is None:
                hpk[0:3, s * 100:s * 100 + 100] = hrows[768:771]
            else:
                hpk[:, s * 100:s * 100 + 100] = hrows[128 * kc:128 * (kc + 1)]
        in_maps.append(dict(
            mf=np.ascontiguousarray(mf_c),
            hsall=hpk.astype(np.float16),
            wall=wall,
            cst=cstm,
        ))
    return in_maps


def kernel(hs, mask_features, references, sizes, W_ctrl, b_ctrl, T):
    assert int(T) == 8
    nc = _get_nc()
    in_maps = _host_pack(hs, mask_features, references, sizes, W_ctrl, b_ctrl)
    res = bass_utils.run_bass_kernel_spmd(nc, in_maps, core_ids=list(range(NCORES)))
    out = np.empty((16, Q, H, W), np.float32)
    for c in range(NCORES):
        out[2 * c:2 * c + 2] = res.results[c]["outp"].reshape(2, Q, H, W)
    return out
